# revision 5
# baseline (speedup 1.0000x reference)
"""Trainium2 Bass kernel for nn_DistributedMoE (moe_routing).

Strategy: pure data-parallel over batch across 8 NeuronCores (128 images each).
Each core runs the routing trunk + gate + all 6 expert CNNs on its slice.
All convs run as shifted-window matmuls on the PE in float32r
(~1.6e-4 rel err, 4x the throughput of plain fp32):
  - conv1 (Cin=3): host-side im2col to K=27 (taps x channels on partitions).
  - conv2 (Cin=32): 3 dh-tap groups packed into K=96. Partition-group order is
    [dh=1, dh=0, dh=2] so the full-data group sits at partition 0 (engines
    cannot shift partitions; the dh=0/dh=2 row-shifted replicas are made by
    SBUF->SBUF DMA, which can).
  - conv3/conv4 (Cin=64): dh {0,1} packed into K=128 (master = fully padded
    buffer at partitions 0:64, dh=1 replica at 64:128) + dh=2 via a +2-row
    free-offset read of the master (K=64).
  - conv5 (Cin=128): 9 taps as free-dim offsets into a padded buffer.
BatchNorm is folded into conv weights/biases on the host; bias+ReLU fuse into
the ScalarE activation pass that drains PSUM; 2x2 maxpools run as
reduce_max (PSUM, w-pairs) + tensor_max (SBUF, h-pairs) on VectorE.
The device returns gate scores [6,B] and per-expert logits [6,10,B]; the tiny
sequential capacity-constrained routing scan + combine run on the host.
"""

import sys

import numpy as np

sys.path.insert(0, "/opt/trn_rl_repo")

from contextlib import ExitStack

import concourse.bacc as bacc
import concourse.bass as bass
import concourse.mybir as mybir
import concourse.tile as tile
from concourse.bass_utils import run_bass_kernel_spmd

# problem constants (from the reference)
E, KTOP, CAP = 6, 2, 192
ALPHA, LOAD_PEN, MIN_USE, TEMP = 0.7, 2.0, 0.05, 1.0
BN_EPS = 1e-5
NEG = -1e30

B = 1024
NCORES = 8
BC = B // NCORES          # images per core
TB = 8                    # images per tile
NIT = BC // TB            # loop iterations per core

F32 = mybir.dt.float32
F32R = mybir.dt.float32r

# dh order for K=96 packing: full-data group (dh=1) first (partition base 0)
DH_ORDER = (1, 0, 2)


# ---------------------------------------------------------------- layouts
def _wlayout():
    """Weight-blob column layout: name -> (rows, col_start, cols)."""
    lay = {}
    col = 0

    def add(name, rows, cols):
        nonlocal col
        lay[name] = (rows, col, cols)
        col += cols

    add("t.c1", 27, 32)
    for dw in range(3):
        add(f"t.c2.{dw}", 96, 32)
    for k in range(4):
        add(f"t.fc.{k}", 32, 64)
    add("t.g1", 64, 32)
    add("t.g2", 32, 6)
    for e in range(E):
        add(f"e{e}.c1", 27, 32)
        for dw in range(3):
            add(f"e{e}.c2.{dw}", 96, 64)
        for dw in range(3):
            add(f"e{e}.c3A.{dw}", 128, 64)
        for dw in range(3):
            add(f"e{e}.c3B.{dw}", 64, 64)
        for dw in range(3):
            add(f"e{e}.c4A.{dw}", 128, 128)
        for dw in range(3):
            add(f"e{e}.c4B.{dw}", 64, 128)
        for dh in range(3):
            for dw in range(3):
                add(f"e{e}.c5.{dh}{dw}", 128, 128)
        add(f"e{e}.fc", 128, 128)
        add(f"e{e}.cls", 128, 10)
    return lay, col


def _blayout():
    lay = {}
    col = 0

    def add(name, rows):
        nonlocal col
        lay[name] = (rows, col)
        col += 1

    add("t.b1", 32)
    add("t.b2", 32)
    add("t.fcb", 64)
    add("t.g1b", 32)
    add("t.g2b", 6)
    for e in range(E):
        for nm, r in [("b1", 32), ("b2", 64), ("b3", 64), ("b4", 128),
                      ("b5", 128), ("fb", 128), ("cb", 10)]:
            add(f"e{e}.{nm}", r)
    return lay, col


WLAY, WCOL = _wlayout()
BLAY, BCOL = _blayout()


# ---------------------------------------------------------------- host prep
def _fold(w, cb, g, bb):
    """Fold conv bias + eval-mode BN into (w', b'). w [co,ci,3,3]."""
    w = np.asarray(w, np.float32)
    g = np.asarray(g, np.float32)
    bb = np.asarray(bb, np.float32)
    s = g / np.sqrt(np.float32(1.0 + BN_EPS))
    wp = w * s[:, None, None, None]
    bp = (np.asarray(cb, np.float32) if cb is not None else 0.0) * s + bb
    return wp.astype(np.float32), np.asarray(bp, np.float32)


def _pack_tap3(wp):
    """[co,ci,3,3] -> lhsT rows (dh*3+dw)*3+ci for the K=27 im2col conv1."""
    return np.ascontiguousarray(wp.transpose(2, 3, 1, 0).reshape(27, wp.shape[0]))


def _pack_96(wp, dw):
    """[co,ci,3,3] -> lhsT [3*ci_n, co] with dh groups ordered DH_ORDER."""
    co, ci = wp.shape[0], wp.shape[1]
    return np.ascontiguousarray(
        wp[:, :, list(DH_ORDER), dw].transpose(2, 1, 0).reshape(3 * ci, co))


def _prep_blobs(inp):
    wb = np.zeros((128, WCOL), np.float32)
    bb = np.zeros((128, BCOL), np.float32)

    def putw(name, arr):
        r, c0, cn = WLAY[name]
        assert arr.shape == (r, cn), (name, arr.shape, (r, cn))
        wb[:r, c0:c0 + cn] = arr

    def putb(name, vec):
        r, c0 = BLAY[name]
        vec = np.asarray(vec, np.float32)
        assert vec.shape == (r,), (name, vec.shape)
        bb[:r, c0] = vec

    # trunk
    w1, b1 = _fold(inp["t_c1w"], None, inp["t_b1g"], inp["t_b1b"])
    putw("t.c1", _pack_tap3(w1)); putb("t.b1", b1)
    w2, b2 = _fold(inp["t_c2w"], None, inp["t_b2g"], inp["t_b2b"])
    for dw in range(3):
        putw(f"t.c2.{dw}", _pack_96(w2, dw))
    putb("t.b2", b2)
    tfcw = np.asarray(inp["t_fcw"], np.float32) / 64.0   # fold avgpool mean
    for k in range(4):
        putw(f"t.fc.{k}", np.ascontiguousarray(tfcw.reshape(64, 32, 4)[:, :, k].T))
    putb("t.fcb", inp["t_fcb"])
    putw("t.g1", np.ascontiguousarray(np.asarray(inp["g1w"], np.float32).T))
    putb("t.g1b", inp["g1b"])
    putw("t.g2", np.ascontiguousarray((np.asarray(inp["g2w"], np.float32) / TEMP).T))
    putb("t.g2b", np.asarray(inp["g2b"], np.float32) / TEMP)

    for e in range(E):
        w1, b1 = _fold(inp["e_c1w"][e], inp["e_c1b"][e], inp["e_b1g"][e], inp["e_b1b"][e])
        putw(f"e{e}.c1", _pack_tap3(w1)); putb(f"e{e}.b1", b1)
        w2, b2 = _fold(inp["e_c2w"][e], inp["e_c2b"][e], inp["e_b2g"][e], inp["e_b2b"][e])
        for dw in range(3):
            putw(f"e{e}.c2.{dw}", _pack_96(w2, dw))
        putb(f"e{e}.b2", b2)
        w3, b3 = _fold(inp["e_c3w"][e], inp["e_c3b"][e], inp["e_b3g"][e], inp["e_b3b"][e])
        for dw in range(3):
            putw(f"e{e}.c3A.{dw}",
                 np.ascontiguousarray(w3[:, :, 0:2, dw].transpose(2, 1, 0).reshape(128, 64)))
            putw(f"e{e}.c3B.{dw}", np.ascontiguousarray(w3[:, :, 2, dw].T))
        putb(f"e{e}.b3", b3)
        w4, b4 = _fold(inp["e_c4w"][e], inp["e_c4b"][e], inp["e_b4g"][e], inp["e_b4b"][e])
        for dw in range(3):
            putw(f"e{e}.c4A.{dw}",
                 np.ascontiguousarray(w4[:, :, 0:2, dw].transpose(2, 1, 0).reshape(128, 128)))
            putw(f"e{e}.c4B.{dw}", np.ascontiguousarray(w4[:, :, 2, dw].T))
        putb(f"e{e}.b4", b4)
        w5, b5 = _fold(inp["e_c5w"][e], inp["e_c5b"][e], inp["e_b5g"][e], inp["e_b5b"][e])
        for dh in range(3):
            for dw in range(3):
                putw(f"e{e}.c5.{dh}{dw}", np.ascontiguousarray(w5[:, :, dh, dw].T))
        putb(f"e{e}.b5", b5)
        putw(f"e{e}.fc",
             np.ascontiguousarray((np.asarray(inp["e_fw"][e], np.float32) / 64.0).T))
        putb(f"e{e}.fb", inp["e_fb"][e])
        putw(f"e{e}.cls", np.ascontiguousarray(np.asarray(inp["e_cw"][e], np.float32).T))
        putb(f"e{e}.cb", inp["e_cb"][e])
    return wb, bb


def _im2col_x(x):
    """x [B,3,32,32] f32 -> [27, B, 1024]: row (dh*3+dw)*3+ci holds
    x[b, ci, h+dh-1, w+dw-1] (zero outside)."""
    Bn = x.shape[0]
    xp = np.zeros((Bn, 3, 34, 34), np.float32)
    xp[:, :, 1:33, 1:33] = x
    out = np.empty((27, Bn, 32, 32), np.float32)
    for dh in range(3):
        for dw in range(3):
            for ci in range(3):
                p = (dh * 3 + dw) * 3 + ci
                out[p] = xp[:, ci, dh:dh + 32, dw:dw + 32]
    return np.ascontiguousarray(out.reshape(27, Bn, 1024))


# ---------------------------------------------------------------- device build
_NC_CACHE = {}


def _build_nc():
    if "nc" in _NC_CACHE:
        return _NC_CACHE["nc"]
    nc = bacc.Bacc("TRN2", target_bir_lowering=False, debug=False)
    x9_d = nc.dram_tensor("x9", [27, BC, 1024], F32R, kind="ExternalInput").ap()
    z_d = nc.dram_tensor("zeros", [128, TB * 32 * 34], F32R, kind="ExternalInput").ap()
    w_d = nc.dram_tensor("wblob", [128, WCOL], F32R, kind="ExternalInput").ap()
    b_d = nc.dram_tensor("bblob", [128, BCOL], F32, kind="ExternalInput").ap()
    gate_d = nc.dram_tensor("gate", [6, BC], F32, kind="ExternalOutput").ap()
    log_d = nc.dram_tensor("logits", [E, 10, BC], F32, kind="ExternalOutput").ap()

    relu = mybir.ActivationFunctionType.Relu
    X = mybir.AxisListType.X

    with tile.TileContext(nc) as tc:
        with ExitStack() as ctx:
            wp = ctx.enter_context(tc.tile_pool(name="wpool", bufs=1))
            ap_ = ctx.enter_context(tc.tile_pool(name="acts", bufs=1))
            tp = ctx.enter_context(tc.tile_pool(name="tmps", bufs=3))
            pp = ctx.enter_context(tc.tile_pool(name="psbig", bufs=5, space="PSUM"))
            ps_ = ctx.enter_context(tc.tile_pool(name="pssmall", bufs=2, space="PSUM"))
            op_ = ctx.enter_context(tc.tile_pool(name="outs", bufs=2))

            wt = wp.tile([128, WCOL], F32R)
            bt = wp.tile([128, BCOL], F32)
            nc.sync.dma_start(wt[:], w_d)
            nc.sync.dma_start(bt[:], b_d)

            def W(name):
                r, c0, cn = WLAY[name]
                return wt[0:r, c0:c0 + cn]

            def BI(name):
                r, c0 = BLAY[name]
                return bt[0:r, c0:c0 + 1]

            xim = ap_.tile([27, TB * 1024], F32R)        # [b,32,32]
            t2in = ap_.tile([96, TB * 32 * 34], F32R)    # [b,32,34] w-padded
            pooled16 = ap_.tile([32, TB * 256], F32)     # [b,16,16]
            avg1 = ap_.tile([32, TB * 32], F32)          # [b,16,2]
            tfeat = ap_.tile([32, 4 * TB], F32R)         # [k(4), b]
            c2in = ap_.tile([96, TB * 288], F32R)        # [b,16,18]
            c3in = ap_.tile([128, TB * 324], F32R)       # [b,18,18]
            c4in = ap_.tile([128, TB * 100], F32R)       # [b,10,10]
            c5in = ap_.tile([128, TB * 100], F32R)       # [b,10,10]
            c5out = ap_.tile([128, TB * 64], F32)        # [b,8,8]
            feat = ap_.tile([128, TB], F32R)
            ffeat = ap_.tile([128, TB], F32R)
            rf = ap_.tile([64, TB], F32R)
            gbuf = ap_.tile([32, TB], F32R)

            # zero the padded buffers once (interiors are rewritten every iter)
            for t_ in (xim, t2in, c2in, c3in, c4in, c5in):
                p_, f_ = t_.shape
                nc.sync.dma_start(t_[:], z_d[0:p_, 0:f_])

            # padded views
            t2v = t2in[:].rearrange("p (b h w) -> p b h w", h=32, w=34)
            p16 = pooled16[:].rearrange("p (b h w) -> p b h w", h=16, w=16)
            a1v = avg1[:].rearrange("p (b h g) -> p b h g", h=16, g=2)
            c2v = c2in[:].rearrange("p (b h w) -> p b h w", h=16, w=18)
            c3v = c3in[:].rearrange("p (b h w) -> p b h w", h=18, w=18)
            c4v = c4in[:].rearrange("p (b h w) -> p b h w", h=10, w=10)
            c5v = c5in[:].rearrange("p (b h w) -> p b h w", h=10, w=10)
            c5o = c5out[:].rearrange("p (b s) -> p b s", s=64)
            ximv = xim[:].rearrange("p (b s) -> p b s", s=1024)

            def body(i):
                # ---- load conv1 im2col tile (one contiguous DMA) ----
                nc.sync.dma_start(ximv, x9_d[:, bass.ds(i * TB, TB), :])

                # ================= trunk =================
                # conv1 -> t2in slot0 (dh=1 = full data rows 0..31, cols 1..33)
                for c in range(2 * TB):
                    ic, hf = c // 2, (c % 2) * 16
                    ps = pp.tile([128, 512], F32, tag="big")
                    nc.tensor.matmul(ps[0:32, :], W("t.c1"),
                                     xim[:, c * 512:(c + 1) * 512],
                                     start=True, stop=True)
                    nc.scalar.activation(
                        t2v[0:32, ic, hf:hf + 16, 1:33],
                        ps[0:32, :].rearrange("p (h w) -> p h w", w=32),
                        relu, bias=BI("t.b1"))
                # replicas: slot1(dh0) rows 1..31 <- slot0 rows 0..30;
                #           slot2(dh2) rows 0..30 <- slot0 rows 1..31
                nc.sync.dma_start(t2v[32:64, :, 1:32, :], t2v[0:32, :, 0:31, :])
                nc.sync.dma_start(t2v[64:96, :, 0:31, :], t2v[0:32, :, 1:32, :])

                # conv2 trunk (K=96 x 3 dw) + maxpool -> pooled16 (raw)
                for c in range(2 * TB):
                    ic, hf = c // 2, (c % 2) * 16
                    ps = pp.tile([128, 512], F32, tag="big")
                    for dw in range(3):
                        nc.tensor.matmul(ps[0:32, :], W(f"t.c2.{dw}"),
                                         t2v[0:96, ic, hf:hf + 16, dw:dw + 32],
                                         start=(dw == 0), stop=(dw == 2))
                    tw = tp.tile([32, 256], F32, tag="tmpw")
                    nc.vector.reduce_max(
                        tw[:].rearrange("p (h w) -> p h w", w=16),
                        ps[0:32, :].rearrange("p (h w t) -> p h w t", w=16, t=2),
                        axis=X)
                    twv = tw[:].rearrange("p (h2 t w) -> p h2 t w", t=2, w=16)
                    nc.vector.tensor_max(
                        p16[:, ic, hf // 2:hf // 2 + 8, :],
                        twv[:, :, 0, :], twv[:, :, 1, :])
                # bias+relu in place, then 8x8 avg (sum; /64 folded into t.fc)
                nc.scalar.activation(pooled16[:], pooled16[:], relu, bias=BI("t.b2"))
                nc.vector.reduce_sum(
                    a1v, p16.rearrange("p b h (g w) -> p b h g w", g=2), axis=X)
                with nc.allow_low_precision(reason="fp32r rounding intended"):
                    nc.vector.reduce_sum(
                        tfeat[:].rearrange("p (i g b) -> p b i g", i=2, g=2),
                        a1v.rearrange("p b (i h) g -> p b i g h", i=2),
                        axis=X)
                # trunk fc (4 accumulating K=32 matmuls) + gate
                psf = ps_.tile([128, TB], F32, tag="small")
                for k in range(4):
                    nc.tensor.matmul(psf[0:64, :], W(f"t.fc.{k}"),
                                     tfeat[:, k * TB:(k + 1) * TB],
                                     start=(k == 0), stop=(k == 3))
                nc.scalar.activation(rf[:], psf[0:64, :], relu, bias=BI("t.fcb"))
                psg = ps_.tile([128, TB], F32, tag="small")
                nc.tensor.matmul(psg[0:32, :], W("t.g1"), rf[:], start=True, stop=True)
                nc.scalar.activation(gbuf[:], psg[0:32, :], relu, bias=BI("t.g1b"))
                psh = ps_.tile([128, TB], F32, tag="small")
                nc.tensor.matmul(psh[0:6, :], W("t.g2"), gbuf[:], start=True, stop=True)
                gs = op_.tile([6, TB], F32, tag="gate")
                nc.vector.tensor_scalar_add(gs[:], psh[0:6, :], BI("t.g2b"))
                nc.sync.dma_start(gate_d[:, bass.ds(i * TB, TB)], gs[:])

                # ================= experts =================
                for e in range(E):
                    # conv1 + pool -> c2in slot0 (rows 0..15 = data) raw
                    for c in range(2 * TB):
                        ic, hf = c // 2, (c % 2) * 16
                        ps = pp.tile([128, 512], F32, tag="big")
                        nc.tensor.matmul(ps[0:32, :], W(f"e{e}.c1"),
                                         xim[:, c * 512:(c + 1) * 512],
                                         start=True, stop=True)
                        tw = tp.tile([32, 256], F32, tag="tmpw")
                        nc.vector.reduce_max(
                            tw[:].rearrange("p (h w) -> p h w", w=16),
                            ps[0:32, :].rearrange("p (h w t) -> p h w t", w=16, t=2),
                            axis=X)
                        twv = tw[:].rearrange("p (h2 t w) -> p h2 t w", t=2, w=16)
                        nc.vector.tensor_max(
                            c2v[0:32, ic, hf // 2:hf // 2 + 8, 1:17],
                            twv[:, :, 0, :], twv[:, :, 1, :])
                    # bias+relu in place on the data region, then replicas
                    nc.scalar.activation(c2v[0:32, :, 0:16, 1:17],
                                         c2v[0:32, :, 0:16, 1:17],
                                         relu, bias=BI(f"e{e}.b1"))
                    nc.sync.dma_start(c2v[32:64, :, 1:16, :], c2v[0:32, :, 0:15, :])
                    nc.sync.dma_start(c2v[64:96, :, 0:15, :], c2v[0:32, :, 1:16, :])

                    # conv2 (K=96 x 3 dw), 2 imgs per chunk -> c3in master interior
                    for cc in range(TB // 2):
                        i0 = 2 * cc
                        ps = pp.tile([128, 512], F32, tag="big")
                        for dw in range(3):
                            nc.tensor.matmul(
                                ps[0:64, :], W(f"e{e}.c2.{dw}"),
                                c2v[0:96, i0:i0 + 2, 0:16, dw:dw + 16],
                                start=(dw == 0), stop=(dw == 2))
                        nc.scalar.activation(
                            c3v[0:64, i0:i0 + 2, 1:17, 1:17],
                            ps[0:64, :].rearrange("p (b h w) -> p b h w", b=2, w=16),
                            relu, bias=BI(f"e{e}.b2"))
                    # dh=1 replica of pad2: rep rows 0..15 <- master rows 1..16
                    nc.sync.dma_start(c3v[64:128, :, 0:16, :], c3v[0:64, :, 1:17, :])

                    # conv3 (A: K=128 dh{0,1}; B: K=64 dh=2 via +2-row offset)
                    # + maxpool -> c4in master interior raw
                    for cc in range(TB // 2):
                        i0 = 2 * cc
                        ps = pp.tile([128, 512], F32, tag="big")
                        for dw in range(3):
                            nc.tensor.matmul(
                                ps[0:64, :], W(f"e{e}.c3A.{dw}"),
                                c3v[0:128, i0:i0 + 2, 0:16, dw:dw + 16],
                                start=(dw == 0), stop=False)
                        for dw in range(3):
                            nc.tensor.matmul(
                                ps[0:64, :], W(f"e{e}.c3B.{dw}"),
                                c3v[0:64, i0:i0 + 2, 2:18, dw:dw + 16],
                                start=False, stop=(dw == 2))
                        t3 = tp.tile([64, 256], F32, tag="tmp3")
                        nc.vector.reduce_max(
                            t3[:].rearrange("p (b h w) -> p b h w", b=2, w=8),
                            ps[0:64, :].rearrange("p (b h w t) -> p b h w t",
                                                  b=2, w=8, t=2),
                            axis=X)
                        t3v = t3[:].rearrange("p (b h2 t w) -> p b h2 t w",
                                              b=2, t=2, w=8)
                        nc.vector.tensor_max(
                            c4v[0:64, i0:i0 + 2, 1:9, 1:9],
                            t3v[:, :, :, 0, :], t3v[:, :, :, 1, :])
                    nc.scalar.activation(c4v[0:64, :, 1:9, 1:9],
                                         c4v[0:64, :, 1:9, 1:9],
                                         relu, bias=BI(f"e{e}.b3"))
                    nc.sync.dma_start(c4v[64:128, :, 0:8, :], c4v[0:64, :, 1:9, :])

                    # conv4 (A: K=128 dh{0,1}; B: K=64 dh=2), one chunk N=512
                    ps4 = pp.tile([128, 512], F32, tag="big")
                    for dw in range(3):
                        nc.tensor.matmul(ps4[:], W(f"e{e}.c4A.{dw}"),
                                         c4v[0:128, :, 0:8, dw:dw + 8],
                                         start=(dw == 0), stop=False)
                    for dw in range(3):
                        nc.tensor.matmul(ps4[:], W(f"e{e}.c4B.{dw}"),
                                         c4v[0:64, :, 2:10, dw:dw + 8],
                                         start=False, stop=(dw == 2))
                    nc.scalar.activation(
                        c5v[:, :, 1:9, 1:9],
                        ps4[:].rearrange("p (b h w) -> p b h w", b=TB, w=8),
                        relu, bias=BI(f"e{e}.b4"))

                    # conv5 (K=128, 9 taps via offsets), one chunk N=512
                    ps5 = pp.tile([128, 512], F32, tag="big")
                    for t9 in range(9):
                        dh, dw = t9 // 3, t9 % 3
                        nc.tensor.matmul(ps5[:], W(f"e{e}.c5.{dh}{dw}"),
                                         c5v[:, :, dh:dh + 8, dw:dw + 8],
                                         start=(t9 == 0), stop=(t9 == 8))
                    nc.scalar.activation(c5o, ps5[:].rearrange("p (b s) -> p b s", s=64),
                                         relu, bias=BI(f"e{e}.b5"))

                    # global average pool (sum; /64 folded into fc) + fc + cls
                    with nc.allow_low_precision(reason="fp32r rounding intended"):
                        nc.vector.reduce_sum(feat[:], c5o, axis=X)
                    psf2 = ps_.tile([128, TB], F32, tag="small")
                    nc.tensor.matmul(psf2[:], W(f"e{e}.fc"), feat[:],
                                     start=True, stop=True)
                    nc.scalar.activation(ffeat[:], psf2[:], relu, bias=BI(f"e{e}.fb"))
                    psc = ps_.tile([128, TB], F32, tag="small")
                    nc.tensor.matmul(psc[0:10, :], W(f"e{e}.cls"), ffeat[:],
                                     start=True, stop=True)
                    lg = op_.tile([10, TB], F32, tag="lg")
                    nc.vector.tensor_scalar_add(lg[:], psc[0:10, :], BI(f"e{e}.cb"))
                    nc.sync.dma_start(log_d[e][:, bass.ds(i * TB, TB)], lg[:])

            with tc.For_i(0, NIT, 1) as i:
                body(i)

    nc.compile()
    _NC_CACHE["nc"] = nc
    return nc


# ---------------------------------------------------------------- host routing
def _route_and_combine(gate_scores, logits_e, ema):
    """Reproduce the reference's conf/balanced/scan/combine on the host."""
    gate = gate_scores.astype(np.float32)
    lg = logits_e.astype(np.float32)
    # conf = -entropy of per-expert softmax
    m = lg.max(axis=2, keepdims=True)
    z = np.exp(lg - m)
    p = z / z.sum(axis=2, keepdims=True)
    logp = np.log(p + 1e-12)
    conf = (p * logp).sum(axis=2).astype(np.float32)          # [B, E]
    combined = (ALPHA * gate + (1.0 - ALPHA) * conf).astype(np.float32)
    ema = np.asarray(ema, np.float32)
    boost = np.where(ema < MIN_USE, (MIN_USE - ema) * 10.0, 0.0).astype(np.float32)
    balanced = (combined + boost[None, :] - LOAD_PEN * ema[None, :]).astype(np.float32)

    topk = np.argsort(-balanced, axis=1, kind="stable")[:, :KTOP]
    loads = np.zeros(E, np.float32)
    chosen = np.zeros(balanced.shape[0], np.int64)
    for i in range(balanced.shape[0]):
        idxs = topk[i]
        l = loads[idxs]
        fits = l < CAP
        c = idxs[int(np.argmax(fits))] if fits.any() else idxs[int(np.argmin(l))]
        loads[c] += 1.0
        chosen[i] = c
    D = np.zeros((balanced.shape[0], E), bool)
    D[np.arange(balanced.shape[0]), chosen] = True

    masked = np.where(D, balanced, np.float32(NEG)).astype(np.float32)
    mm = masked.max(axis=1, keepdims=True)
    w = np.exp(masked - mm)
    w = w / w.sum(axis=1, keepdims=True)
    w = (w * D).astype(np.float32)
    w = w / (w.sum(axis=1, keepdims=True) + 1e-12)
    logits_final = (w[:, :, None] * lg).sum(axis=1).astype(np.float32)
    return logits_final, balanced, D


def _run_device(inp, trace=False):
    wb, bb = _prep_blobs(inp)
    x9 = _im2col_x(np.asarray(inp["x"], np.float32))
    nc = _build_nc()
    in_maps = []
    for c in range(NCORES):
        in_maps.append({
            "x9": np.ascontiguousarray(x9[:, c * BC:(c + 1) * BC, :]),
            "wblob": wb,
            "bblob": bb,
            "zeros": np.zeros((128, TB * 32 * 34), np.float32),
        })
    res = run_bass_kernel_spmd(nc, in_maps, core_ids=list(range(NCORES)),
                               trace=trace)
    gate_scores = np.empty((B, E), np.float32)
    logits_e = np.empty((B, E, 10), np.float32)
    for c in range(NCORES):
        r = res.results[c]
        gate_scores[c * BC:(c + 1) * BC, :] = r["gate"].T
        logits_e[c * BC:(c + 1) * BC, :, :] = r["logits"].transpose(2, 0, 1)
    return gate_scores, logits_e, res


def kernel(**inputs):
    gate_scores, logits_e, _ = _run_device(inputs, trace=False)
    return _route_and_combine(gate_scores, logits_e, inputs["ema"])


def _install_ntff_hook():
    """Provide antenv.axon_hooks (absent in this image) so trace=True works."""
    import types
    try:
        import antenv.axon_hooks  # noqa: F401
        return
    except ImportError:
        pass
    try:
        import antenv
        sys.path.insert(0, "/root/.axon_site")
        from trn_agent_boot.trn_boot import _ntff_profile_via_ctypes
        mod = types.ModuleType("antenv.axon_hooks")
        _h = [None]
        mod.set_axon_ntff_profile_hook = lambda h: _h.__setitem__(0, h)
        mod.get_axon_ntff_profile_hook = lambda: _h[0]
        sys.modules["antenv.axon_hooks"] = mod
        antenv.axon_hooks = mod
        mod.set_axon_ntff_profile_hook(
            _ntff_profile_via_ctypes("/opt/axon/libaxon_pjrt.so"))
    except Exception as ex:  # profiling is best-effort
        print("ntff hook install failed:", ex)


def kernel_with_stats(**inputs):
    """Like kernel() but traces the run and returns (outputs, exec_time_ns)."""
    _install_ntff_hook()
    gate_scores, logits_e, res = _run_device(inputs, trace=True)
    out = _route_and_combine(gate_scores, logits_e, inputs["ema"])
    return out, res.exec_time_ns


# revision 12
# speedup vs baseline: 1.2828x; 1.2828x over previous
"""Trainium2 Bass kernel for nn_DistributedMoE (moe_routing).

Strategy: pure data-parallel over batch across 8 NeuronCores (128 images each).
Each core runs the routing trunk + gate + all 6 expert CNNs on its slice.
All convs run as shifted-window matmuls on the PE in float32r
(~1.6e-4 rel err, 4x the throughput of plain fp32):
  - conv1 (Cin=3): host-side im2col to K=27 (taps x channels on partitions).
  - conv2 (Cin=32): 3 dh-tap groups packed into K=96. Partition-group order is
    [dh=1, dh=0, dh=2] so the full-data group sits at partition 0 (engines
    cannot shift partitions; the dh=0/dh=2 row-shifted replicas are made by
    SBUF->SBUF DMA, which can).
  - conv3/conv4 (Cin=64): dh {0,1} packed into K=128 (master = fully padded
    buffer at partitions 0:64, dh=1 replica at 64:128) + dh=2 via a +2-row
    free-offset read of the master (K=64).
  - conv5 (Cin=128): 9 taps as free-dim offsets into a padded buffer.
BatchNorm is folded into conv weights/biases on the host; bias+ReLU fuse into
the ScalarE activation pass that drains PSUM; 2x2 maxpools run as
reduce_max (PSUM, w-pairs) + tensor_max (SBUF, h-pairs) on VectorE.
The device returns gate scores [6,B] and per-expert logits [6,10,B]; the tiny
sequential capacity-constrained routing scan + combine run on the host.
"""

import sys

import numpy as np

sys.path.insert(0, "/opt/trn_rl_repo")

from contextlib import ExitStack

import concourse.bacc as bacc
import concourse.bass as bass
import concourse.mybir as mybir
import concourse.tile as tile
from concourse.bass_utils import run_bass_kernel_spmd

# problem constants (from the reference)
E, KTOP, CAP = 6, 2, 192
ALPHA, LOAD_PEN, MIN_USE, TEMP = 0.7, 2.0, 0.05, 1.0
BN_EPS = 1e-5
NEG = -1e30

B = 1024
NCORES = 8
BC = B // NCORES          # images per core
TB = 8                    # images per tile
NIT = BC // TB            # loop iterations per core

F32 = mybir.dt.float32
F32R = mybir.dt.float32r

# dh order for K=96 packing: full-data group (dh=1) first (partition base 0)
DH_ORDER = (1, 0, 2)


# ---------------------------------------------------------------- layouts
def _wlayout():
    """Weight-blob column layout: name -> (rows, col_start, cols)."""
    lay = {}
    col = 0

    def add(name, rows, cols):
        nonlocal col
        lay[name] = (rows, col, cols)
        col += cols

    add("t.c1", 27, 32)
    for dw in range(3):
        add(f"t.c2.{dw}", 96, 32)
    for k in range(4):
        add(f"t.fc.{k}", 32, 64)
    add("t.g1", 64, 32)
    add("t.g2", 32, 6)
    for e in range(E):
        add(f"e{e}.c1", 27, 32)
        for dw in range(3):
            add(f"e{e}.c2.{dw}", 96, 64)
        for dw in range(3):
            add(f"e{e}.c3A.{dw}", 128, 64)
        for dw in range(3):
            add(f"e{e}.c3B.{dw}", 64, 64)
        for dw in range(3):
            add(f"e{e}.c4A.{dw}", 128, 128)
        for dw in range(3):
            add(f"e{e}.c4B.{dw}", 64, 128)
        for dh in range(3):
            for dw in range(3):
                add(f"e{e}.c5.{dh}{dw}", 128, 128)
        add(f"e{e}.fc", 128, 128)
        add(f"e{e}.cls", 128, 10)
    return lay, col


def _blayout():
    lay = {}
    col = 0

    def add(name, rows):
        nonlocal col
        lay[name] = (rows, col)
        col += 1

    add("t.b1", 32)
    add("t.b2", 32)
    add("t.fcb", 64)
    add("t.g1b", 32)
    add("t.g2b", 6)
    for e in range(E):
        for nm, r in [("b1", 32), ("b2", 64), ("b3", 64), ("b4", 128),
                      ("b5", 128), ("fb", 128), ("cb", 10)]:
            add(f"e{e}.{nm}", r)
    return lay, col


WLAY, WCOL = _wlayout()
BLAY, BCOL = _blayout()


# ---------------------------------------------------------------- host prep
def _fold(w, cb, g, bb):
    """Fold conv bias + eval-mode BN into (w', b'). w [co,ci,3,3]."""
    w = np.asarray(w, np.float32)
    g = np.asarray(g, np.float32)
    bb = np.asarray(bb, np.float32)
    s = g / np.sqrt(np.float32(1.0 + BN_EPS))
    wp = w * s[:, None, None, None]
    bp = (np.asarray(cb, np.float32) if cb is not None else 0.0) * s + bb
    return wp.astype(np.float32), np.asarray(bp, np.float32)


def _pack_tap3(wp):
    """[co,ci,3,3] -> lhsT rows (dh*3+dw)*3+ci for the K=27 im2col conv1."""
    return np.ascontiguousarray(wp.transpose(2, 3, 1, 0).reshape(27, wp.shape[0]))


def _pack_96(wp, dw):
    """[co,ci,3,3] -> lhsT [3*ci_n, co] with dh groups ordered DH_ORDER."""
    co, ci = wp.shape[0], wp.shape[1]
    return np.ascontiguousarray(
        wp[:, :, list(DH_ORDER), dw].transpose(2, 1, 0).reshape(3 * ci, co))


def _prep_blobs(inp):
    wb = np.zeros((128, WCOL), np.float32)
    bb = np.zeros((128, BCOL), np.float32)

    def putw(name, arr):
        r, c0, cn = WLAY[name]
        assert arr.shape == (r, cn), (name, arr.shape, (r, cn))
        wb[:r, c0:c0 + cn] = arr

    def putb(name, vec):
        r, c0 = BLAY[name]
        vec = np.asarray(vec, np.float32)
        assert vec.shape == (r,), (name, vec.shape)
        bb[:r, c0] = vec

    # trunk
    w1, b1 = _fold(inp["t_c1w"], None, inp["t_b1g"], inp["t_b1b"])
    putw("t.c1", _pack_tap3(w1)); putb("t.b1", b1)
    w2, b2 = _fold(inp["t_c2w"], None, inp["t_b2g"], inp["t_b2b"])
    for dw in range(3):
        putw(f"t.c2.{dw}", _pack_96(w2, dw))
    putb("t.b2", b2)
    tfcw = np.asarray(inp["t_fcw"], np.float32) / 64.0   # fold avgpool mean
    for k in range(4):
        putw(f"t.fc.{k}", np.ascontiguousarray(tfcw.reshape(64, 32, 4)[:, :, k].T))
    putb("t.fcb", inp["t_fcb"])
    putw("t.g1", np.ascontiguousarray(np.asarray(inp["g1w"], np.float32).T))
    putb("t.g1b", inp["g1b"])
    putw("t.g2", np.ascontiguousarray((np.asarray(inp["g2w"], np.float32) / TEMP).T))
    putb("t.g2b", np.asarray(inp["g2b"], np.float32) / TEMP)

    for e in range(E):
        w1, b1 = _fold(inp["e_c1w"][e], inp["e_c1b"][e], inp["e_b1g"][e], inp["e_b1b"][e])
        putw(f"e{e}.c1", _pack_tap3(w1)); putb(f"e{e}.b1", b1)
        w2, b2 = _fold(inp["e_c2w"][e], inp["e_c2b"][e], inp["e_b2g"][e], inp["e_b2b"][e])
        for dw in range(3):
            putw(f"e{e}.c2.{dw}", _pack_96(w2, dw))
        putb(f"e{e}.b2", b2)
        w3, b3 = _fold(inp["e_c3w"][e], inp["e_c3b"][e], inp["e_b3g"][e], inp["e_b3b"][e])
        for dw in range(3):
            putw(f"e{e}.c3A.{dw}",
                 np.ascontiguousarray(w3[:, :, 0:2, dw].transpose(2, 1, 0).reshape(128, 64)))
            putw(f"e{e}.c3B.{dw}", np.ascontiguousarray(w3[:, :, 2, dw].T))
        putb(f"e{e}.b3", b3)
        w4, b4 = _fold(inp["e_c4w"][e], inp["e_c4b"][e], inp["e_b4g"][e], inp["e_b4b"][e])
        for dw in range(3):
            putw(f"e{e}.c4A.{dw}",
                 np.ascontiguousarray(w4[:, :, 0:2, dw].transpose(2, 1, 0).reshape(128, 128)))
            putw(f"e{e}.c4B.{dw}", np.ascontiguousarray(w4[:, :, 2, dw].T))
        putb(f"e{e}.b4", b4)
        w5, b5 = _fold(inp["e_c5w"][e], inp["e_c5b"][e], inp["e_b5g"][e], inp["e_b5b"][e])
        for dh in range(3):
            for dw in range(3):
                putw(f"e{e}.c5.{dh}{dw}", np.ascontiguousarray(w5[:, :, dh, dw].T))
        putb(f"e{e}.b5", b5)
        putw(f"e{e}.fc",
             np.ascontiguousarray((np.asarray(inp["e_fw"][e], np.float32) / 64.0).T))
        putb(f"e{e}.fb", inp["e_fb"][e])
        putw(f"e{e}.cls", np.ascontiguousarray(np.asarray(inp["e_cw"][e], np.float32).T))
        putb(f"e{e}.cb", inp["e_cb"][e])
    return wb, bb


def _im2col_x(x):
    """x [B,3,32,32] f32 -> [27, B, 1024]: row (dh*3+dw)*3+ci holds
    x[b, ci, h+dh-1, w+dw-1] (zero outside)."""
    Bn = x.shape[0]
    xp = np.zeros((Bn, 3, 34, 34), np.float32)
    xp[:, :, 1:33, 1:33] = x
    out = np.empty((27, Bn, 32, 32), np.float32)
    for dh in range(3):
        for dw in range(3):
            for ci in range(3):
                p = (dh * 3 + dw) * 3 + ci
                out[p] = xp[:, ci, dh:dh + 32, dw:dw + 32]
    return np.ascontiguousarray(out.reshape(27, Bn, 1024))


# ---------------------------------------------------------------- device build
_NC_CACHE = {}


def _build_nc():
    if "nc" in _NC_CACHE:
        return _NC_CACHE["nc"]
    nc = bacc.Bacc("TRN2", target_bir_lowering=False, debug=False)
    x9_d = nc.dram_tensor("x9", [27, BC, 1024], F32R, kind="ExternalInput").ap()
    z_d = nc.dram_tensor("zeros", [128, TB * 32 * 34], F32R, kind="ExternalInput").ap()
    w_d = nc.dram_tensor("wblob", [128, WCOL], F32R, kind="ExternalInput").ap()
    b_d = nc.dram_tensor("bblob", [128, BCOL], F32, kind="ExternalInput").ap()
    gate_d = nc.dram_tensor("gate", [6, BC], F32, kind="ExternalOutput").ap()
    log_d = nc.dram_tensor("logits", [E, 10, BC], F32, kind="ExternalOutput").ap()

    relu = mybir.ActivationFunctionType.Relu
    X = mybir.AxisListType.X
    XY = mybir.AxisListType.XY

    with tile.TileContext(nc) as tc:
        with ExitStack() as ctx:
            wp = ctx.enter_context(tc.tile_pool(name="wpool", bufs=1))
            ap_ = ctx.enter_context(tc.tile_pool(name="acts", bufs=1))
            pp = ctx.enter_context(tc.tile_pool(name="psbig", bufs=6, space="PSUM"))
            ps_ = ctx.enter_context(tc.tile_pool(name="pssmall", bufs=2, space="PSUM"))
            op_ = ctx.enter_context(tc.tile_pool(name="outs", bufs=2))

            wt = wp.tile([128, WCOL], F32R)
            bt = wp.tile([128, BCOL], F32)
            nc.sync.dma_start(wt[:], w_d)
            nc.sync.dma_start(bt[:], b_d)

            def W(name):
                r, c0, cn = WLAY[name]
                return wt[0:r, c0:c0 + cn]

            def BI(name):
                r, c0 = BLAY[name]
                return bt[0:r, c0:c0 + 1]

            xim = ap_.tile([27, TB * 1024], F32R)        # [b,32,32]
            t2in = ap_.tile([96, TB * 32 * 34], F32R)    # [b,32,34] w-padded
            pooled16 = ap_.tile([32, TB * 256], F32)     # [b,16,16]
            avg1 = ap_.tile([32, TB * 32], F32)          # [b,16,2]
            tfeat = ap_.tile([32, 4 * TB], F32R)         # [k(4), b]
            c2a = ap_.tile([96, TB * 288], F32R)         # [b,16,18] ping
            c2b = ap_.tile([96, TB * 288], F32R)         # [b,16,18] pong
            c3in = ap_.tile([128, TB * 324], F32R)       # [b,18,18]
            c4in = ap_.tile([128, TB * 100], F32R)       # [b,10,10]
            c5in = ap_.tile([128, TB * 100], F32R)       # [b,10,10]
            c5out = ap_.tile([128, TB * 64], F32)        # [b,8,8]
            feat = ap_.tile([128, TB], F32R)
            ffeat = ap_.tile([128, TB], F32R)
            rf = ap_.tile([64, TB], F32R)
            gbuf = ap_.tile([32, TB], F32R)

            # zero the padded buffers once (interiors are rewritten every iter)
            for t_ in (xim, t2in, c2a, c2b, c3in, c4in, c5in):
                p_, f_ = t_.shape
                nc.sync.dma_start(t_[:], z_d[0:p_, 0:f_])

            # padded views
            t2v = t2in[:].rearrange("p (b h w) -> p b h w", h=32, w=34)
            p16 = pooled16[:].rearrange("p (b h w) -> p b h w", h=16, w=16)
            a1v = avg1[:].rearrange("p (b h g) -> p b h g", h=16, g=2)
            c2vs = [t[:].rearrange("p (b h w) -> p b h w", h=16, w=18)
                    for t in (c2a, c2b)]
            c3v = c3in[:].rearrange("p (b h w) -> p b h w", h=18, w=18)
            c4v = c4in[:].rearrange("p (b h w) -> p b h w", h=10, w=10)
            c5v = c5in[:].rearrange("p (b h w) -> p b h w", h=10, w=10)
            c5o = c5out[:].rearrange("p (b s) -> p b s", s=64)
            ximv = xim[:].rearrange("p (b s) -> p b s", s=1024)

            def pool2x2(dst, ps, parts):
                """One-op 2x2 maxpool of a [parts, 16x32] psum chunk."""
                v = ps[0:parts, :].rearrange(
                    "p (ho wh wo ww) -> p ho wo wh ww", ho=8, wh=2, ww=2)
                nc.vector.reduce_max(dst, v, axis=XY)

            def conv1_expert(i, e, c2v):
                """Expert conv1 + fused 2x2 maxpool -> c2v slot0 raw interior."""
                for c in range(2 * TB):
                    ic, hf = c // 2, (c % 2) * 16
                    ps = pp.tile([128, 512], F32, tag="big")
                    nc.tensor.matmul(ps[0:32, :], W(f"e{e}.c1"),
                                     xim[:, c * 512:(c + 1) * 512],
                                     start=True, stop=True)
                    pool2x2(c2v[0:32, ic, hf // 2:hf // 2 + 8, 1:17], ps, 32)

            def trunk_conv1(i):
                for c in range(2 * TB):
                    ic, hf = c // 2, (c % 2) * 16
                    ps = pp.tile([128, 512], F32, tag="big")
                    nc.tensor.matmul(ps[0:32, :], W("t.c1"),
                                     xim[:, c * 512:(c + 1) * 512],
                                     start=True, stop=True)
                    nc.scalar.activation(
                        t2v[0:32, ic, hf:hf + 16, 1:33],
                        ps[0:32, :].rearrange("p (h w) -> p h w", w=32),
                        relu, bias=BI("t.b1"))
                nc.sync.dma_start(t2v[32:64, :, 1:32, :], t2v[0:32, :, 0:31, :])
                nc.sync.dma_start(t2v[64:96, :, 0:31, :], t2v[0:32, :, 1:32, :])

            def trunk_rest(i):
                # conv2 (K=96 x 3 dw) + 1-op maxpool -> pooled16 (raw)
                for c in range(2 * TB):
                    ic, hf = c // 2, (c % 2) * 16
                    ps = pp.tile([128, 512], F32, tag="big")
                    for dw in range(3):
                        nc.tensor.matmul(ps[0:32, :], W(f"t.c2.{dw}"),
                                         t2v[0:96, ic, hf:hf + 16, dw:dw + 32],
                                         start=(dw == 0), stop=(dw == 2))
                    pool2x2(p16[:, ic, hf // 2:hf // 2 + 8, :], ps, 32)
                nc.scalar.activation(pooled16[:], pooled16[:], relu, bias=BI("t.b2"))
                nc.vector.reduce_sum(
                    a1v, p16.rearrange("p b h (g w) -> p b h g w", g=2), axis=X)
                with nc.allow_low_precision(reason="fp32r rounding intended"):
                    nc.vector.reduce_sum(
                        tfeat[:].rearrange("p (i g b) -> p b i g", i=2, g=2),
                        a1v.rearrange("p b (i h) g -> p b i g h", i=2),
                        axis=X)
                psf = ps_.tile([128, TB], F32, tag="small")
                for k in range(4):
                    nc.tensor.matmul(psf[0:64, :], W(f"t.fc.{k}"),
                                     tfeat[:, k * TB:(k + 1) * TB],
                                     start=(k == 0), stop=(k == 3))
                nc.scalar.activation(rf[:], psf[0:64, :], relu, bias=BI("t.fcb"))
                psg = ps_.tile([128, TB], F32, tag="small")
                nc.tensor.matmul(psg[0:32, :], W("t.g1"), rf[:], start=True, stop=True)
                nc.scalar.activation(gbuf[:], psg[0:32, :], relu, bias=BI("t.g1b"))
                psh = ps_.tile([128, TB], F32, tag="small")
                nc.tensor.matmul(psh[0:6, :], W("t.g2"), gbuf[:], start=True, stop=True)
                gs = op_.tile([6, TB], F32, tag="gate")
                nc.vector.tensor_scalar_add(gs[:], psh[0:6, :], BI("t.g2b"))
                nc.sync.dma_start(gate_d[:, bass.ds(i * TB, TB)], gs[:])

            def expert_rest(i, e, c2v):
                """bias/relu + replicas on c2v, then conv2..cls for expert e."""
                nc.scalar.activation(c2v[0:32, :, 0:16, 1:17],
                                     c2v[0:32, :, 0:16, 1:17],
                                     relu, bias=BI(f"e{e}.b1"))
                nc.sync.dma_start(c2v[32:64, :, 1:16, :], c2v[0:32, :, 0:15, :])
                nc.sync.dma_start(c2v[64:96, :, 0:15, :], c2v[0:32, :, 1:16, :])

                # conv2 (K=96 x 3 dw), 2 imgs per chunk -> c3in master interior
                for cc in range(TB // 2):
                    i0 = 2 * cc
                    ps = pp.tile([128, 512], F32, tag="big")
                    for dw in range(3):
                        nc.tensor.matmul(
                            ps[0:64, :], W(f"e{e}.c2.{dw}"),
                            c2v[0:96, i0:i0 + 2, 0:16, dw:dw + 16],
                            start=(dw == 0), stop=(dw == 2))
                    nc.scalar.activation(
                        c3v[0:64, i0:i0 + 2, 1:17, 1:17],
                        ps[0:64, :].rearrange("p (b h w) -> p b h w", b=2, w=16),
                        relu, bias=BI(f"e{e}.b2"))
                # dh=1 replica of pad2: rep rows 0..15 <- master rows 1..16
                nc.sync.dma_start(c3v[64:128, :, 0:16, :], c3v[0:64, :, 1:17, :])

                # conv3 (A: K=128 dh{0,1}; B: K=64 dh=2 via +2-row offset)
                # + 1-op maxpool -> c4in master interior raw
                for cc in range(TB // 2):
                    i0 = 2 * cc
                    ps = pp.tile([128, 512], F32, tag="big")
                    for dw in range(3):
                        nc.tensor.matmul(
                            ps[0:64, :], W(f"e{e}.c3A.{dw}"),
                            c3v[0:128, i0:i0 + 2, 0:16, dw:dw + 16],
                            start=(dw == 0), stop=False)
                    for dw in range(3):
                        nc.tensor.matmul(
                            ps[0:64, :], W(f"e{e}.c3B.{dw}"),
                            c3v[0:64, i0:i0 + 2, 2:18, dw:dw + 16],
                            start=False, stop=(dw == 2))
                    # chunk is [64, 2 imgs x 16 x 16]: pool each img
                    v = ps[0:64, :].rearrange(
                        "p (b ho wh wo ww) -> p b ho wo wh ww",
                        b=2, ho=8, wh=2, ww=2)
                    nc.vector.reduce_max(c4v[0:64, i0:i0 + 2, 1:9, 1:9], v, axis=XY)
                nc.scalar.activation(c4v[0:64, :, 1:9, 1:9],
                                     c4v[0:64, :, 1:9, 1:9],
                                     relu, bias=BI(f"e{e}.b3"))
                nc.sync.dma_start(c4v[64:128, :, 0:8, :], c4v[0:64, :, 1:9, :])

                # conv4 (A: K=128 dh{0,1}; B: K=64 dh=2), one chunk N=512
                ps4 = pp.tile([128, 512], F32, tag="big")
                for dw in range(3):
                    nc.tensor.matmul(ps4[:], W(f"e{e}.c4A.{dw}"),
                                     c4v[0:128, :, 0:8, dw:dw + 8],
                                     start=(dw == 0), stop=False)
                for dw in range(3):
                    nc.tensor.matmul(ps4[:], W(f"e{e}.c4B.{dw}"),
                                     c4v[0:64, :, 2:10, dw:dw + 8],
                                     start=False, stop=(dw == 2))
                nc.scalar.activation(
                    c5v[:, :, 1:9, 1:9],
                    ps4[:].rearrange("p (b h w) -> p b h w", b=TB, w=8),
                    relu, bias=BI(f"e{e}.b4"))

                # conv5 (K=128, 9 taps via offsets), one chunk N=512
                ps5 = pp.tile([128, 512], F32, tag="big")
                for t9 in range(9):
                    dh, dw = t9 // 3, t9 % 3
                    nc.tensor.matmul(ps5[:], W(f"e{e}.c5.{dh}{dw}"),
                                     c5v[:, :, dh:dh + 8, dw:dw + 8],
                                     start=(t9 == 0), stop=(t9 == 8))
                nc.scalar.activation(c5o, ps5[:].rearrange("p (b s) -> p b s", s=64),
                                     relu, bias=BI(f"e{e}.b5"))

                # global average pool (sum; /64 folded into fc) + fc + cls
                with nc.allow_low_precision(reason="fp32r rounding intended"):
                    nc.vector.reduce_sum(feat[:], c5o, axis=X)
                psf2 = ps_.tile([128, TB], F32, tag="small")
                nc.tensor.matmul(psf2[:], W(f"e{e}.fc"), feat[:],
                                 start=True, stop=True)
                nc.scalar.activation(ffeat[:], psf2[:], relu, bias=BI(f"e{e}.fb"))
                psc = ps_.tile([128, TB], F32, tag="small")
                nc.tensor.matmul(psc[0:10, :], W(f"e{e}.cls"), ffeat[:],
                                 start=True, stop=True)
                lg = op_.tile([10, TB], F32, tag="lg")
                nc.vector.tensor_scalar_add(lg[:], psc[0:10, :], BI(f"e{e}.cb"))
                nc.sync.dma_start(log_d[e][:, bass.ds(i * TB, TB)], lg[:])

            def body(i):
                nc.sync.dma_start(ximv, x9_d[:, bass.ds(i * TB, TB), :])
                # software pipeline: conv1(e+1) overlaps expert e's conv2..cls
                conv1_expert(i, 0, c2vs[0])
                trunk_conv1(i)
                for e in range(E):
                    if e + 1 < E:
                        conv1_expert(i, e + 1, c2vs[(e + 1) % 2])
                    expert_rest(i, e, c2vs[e % 2])
                # trunk conv2/gate last: filler PE work for scheduling slack
                trunk_rest(i)

            with tc.For_i(0, NIT, 1) as i:
                body(i)

    nc.compile()
    _NC_CACHE["nc"] = nc
    return nc


# ---------------------------------------------------------------- host routing
def _route_and_combine(gate_scores, logits_e, ema):
    """Reproduce the reference's conf/balanced/scan/combine on the host."""
    gate = gate_scores.astype(np.float32)
    lg = logits_e.astype(np.float32)
    # conf = -entropy of per-expert softmax
    m = lg.max(axis=2, keepdims=True)
    z = np.exp(lg - m)
    p = z / z.sum(axis=2, keepdims=True)
    logp = np.log(p + 1e-12)
    conf = (p * logp).sum(axis=2).astype(np.float32)          # [B, E]
    combined = (ALPHA * gate + (1.0 - ALPHA) * conf).astype(np.float32)
    ema = np.asarray(ema, np.float32)
    boost = np.where(ema < MIN_USE, (MIN_USE - ema) * 10.0, 0.0).astype(np.float32)
    balanced = (combined + boost[None, :] - LOAD_PEN * ema[None, :]).astype(np.float32)

    topk = np.argsort(-balanced, axis=1, kind="stable")[:, :KTOP]
    loads = np.zeros(E, np.float32)
    chosen = np.zeros(balanced.shape[0], np.int64)
    for i in range(balanced.shape[0]):
        idxs = topk[i]
        l = loads[idxs]
        fits = l < CAP
        c = idxs[int(np.argmax(fits))] if fits.any() else idxs[int(np.argmin(l))]
        loads[c] += 1.0
        chosen[i] = c
    D = np.zeros((balanced.shape[0], E), bool)
    D[np.arange(balanced.shape[0]), chosen] = True

    masked = np.where(D, balanced, np.float32(NEG)).astype(np.float32)
    mm = masked.max(axis=1, keepdims=True)
    w = np.exp(masked - mm)
    w = w / w.sum(axis=1, keepdims=True)
    w = (w * D).astype(np.float32)
    w = w / (w.sum(axis=1, keepdims=True) + 1e-12)
    logits_final = (w[:, :, None] * lg).sum(axis=1).astype(np.float32)
    return logits_final, balanced, D


def _run_device(inp, trace=False):
    wb, bb = _prep_blobs(inp)
    x9 = _im2col_x(np.asarray(inp["x"], np.float32))
    nc = _build_nc()
    in_maps = []
    for c in range(NCORES):
        in_maps.append({
            "x9": np.ascontiguousarray(x9[:, c * BC:(c + 1) * BC, :]),
            "wblob": wb,
            "bblob": bb,
            "zeros": np.zeros((128, TB * 32 * 34), np.float32),
        })
    res = run_bass_kernel_spmd(nc, in_maps, core_ids=list(range(NCORES)),
                               trace=trace)
    gate_scores = np.empty((B, E), np.float32)
    logits_e = np.empty((B, E, 10), np.float32)
    for c in range(NCORES):
        r = res.results[c]
        gate_scores[c * BC:(c + 1) * BC, :] = r["gate"].T
        logits_e[c * BC:(c + 1) * BC, :, :] = r["logits"].transpose(2, 0, 1)
    return gate_scores, logits_e, res


def kernel(**inputs):
    gate_scores, logits_e, _ = _run_device(inputs, trace=False)
    return _route_and_combine(gate_scores, logits_e, inputs["ema"])


def _install_ntff_hook():
    """Provide antenv.axon_hooks (absent in this image) so trace=True works."""
    import types
    try:
        import antenv.axon_hooks  # noqa: F401
        return
    except ImportError:
        pass
    try:
        import antenv
        sys.path.insert(0, "/root/.axon_site")
        from trn_agent_boot.trn_boot import _ntff_profile_via_ctypes
        mod = types.ModuleType("antenv.axon_hooks")
        _h = [None]
        mod.set_axon_ntff_profile_hook = lambda h: _h.__setitem__(0, h)
        mod.get_axon_ntff_profile_hook = lambda: _h[0]
        sys.modules["antenv.axon_hooks"] = mod
        antenv.axon_hooks = mod
        mod.set_axon_ntff_profile_hook(
            _ntff_profile_via_ctypes("/opt/axon/libaxon_pjrt.so"))
    except Exception as ex:  # profiling is best-effort
        print("ntff hook install failed:", ex)


def kernel_with_stats(**inputs):
    """Like kernel() but traces the run and returns (outputs, exec_time_ns)."""
    _install_ntff_hook()
    gate_scores, logits_e, res = _run_device(inputs, trace=True)
    out = _route_and_combine(gate_scores, logits_e, inputs["ema"])
    return out, res.exec_time_ns


# revision 13
# speedup vs baseline: 1.3110x; 1.0219x over previous
"""Trainium2 Bass kernel for nn_DistributedMoE (moe_routing).

Strategy: pure data-parallel over batch across 8 NeuronCores (128 images each).
Each core runs the routing trunk + gate + all 6 expert CNNs on its slice.
All convs run as shifted-window matmuls on the PE in float32r
(~1.6e-4 rel err, 4x the throughput of plain fp32):
  - conv1 (Cin=3): host-side im2col to K=27 (taps x channels on partitions).
  - conv2 (Cin=32): 3 dh-tap groups packed into K=96. Partition-group order is
    [dh=1, dh=0, dh=2] so the full-data group sits at partition 0 (engines
    cannot shift partitions; the dh=0/dh=2 row-shifted replicas are made by
    SBUF->SBUF DMA, which can).
  - conv3/conv4 (Cin=64): dh {0,1} packed into K=128 (master = fully padded
    buffer at partitions 0:64, dh=1 replica at 64:128) + dh=2 via a +2-row
    free-offset read of the master (K=64).
  - conv5 (Cin=128): 9 taps as free-dim offsets into a padded buffer.
BatchNorm is folded into conv weights/biases on the host; bias+ReLU fuse into
the ScalarE activation pass that drains PSUM; 2x2 maxpools run as
reduce_max (PSUM, w-pairs) + tensor_max (SBUF, h-pairs) on VectorE.
The device returns gate scores [6,B] and per-expert logits [6,10,B]; the tiny
sequential capacity-constrained routing scan + combine run on the host.
"""

import sys

import numpy as np

sys.path.insert(0, "/opt/trn_rl_repo")

from contextlib import ExitStack

import concourse.bacc as bacc
import concourse.bass as bass
import concourse.mybir as mybir
import concourse.tile as tile
from concourse.bass_utils import run_bass_kernel_spmd

# problem constants (from the reference)
E, KTOP, CAP = 6, 2, 192
ALPHA, LOAD_PEN, MIN_USE, TEMP = 0.7, 2.0, 0.05, 1.0
BN_EPS = 1e-5
NEG = -1e30

B = 1024
NCORES = 8
BC = B // NCORES          # images per core
TB = 8                    # images per tile
NIT = BC // TB            # loop iterations per core

F32 = mybir.dt.float32
F32R = mybir.dt.float32r

# dh order for K=96 packing: full-data group (dh=1) first (partition base 0)
DH_ORDER = (1, 0, 2)


# ---------------------------------------------------------------- layouts
def _wlayout():
    """Weight-blob column layout: name -> (rows, col_start, cols)."""
    lay = {}
    col = 0

    def add(name, rows, cols):
        nonlocal col
        lay[name] = (rows, col, cols)
        col += cols

    add("t.c1", 27, 32)
    for dw in range(3):
        add(f"t.c2.{dw}", 96, 32)
    for k in range(4):
        add(f"t.fc.{k}", 32, 64)
    add("t.g1", 64, 32)
    add("t.g2", 32, 6)
    for e in range(E):
        add(f"e{e}.c1", 27, 32)
        for dw in range(3):
            add(f"e{e}.c2.{dw}", 96, 64)
        for dw in range(3):
            add(f"e{e}.c3A.{dw}", 128, 64)
        for dw in range(3):
            add(f"e{e}.c3B.{dw}", 64, 64)
        for dw in range(3):
            add(f"e{e}.c4A.{dw}", 128, 128)
        for dw in range(3):
            add(f"e{e}.c4B.{dw}", 64, 128)
        for dh in range(3):
            for dw in range(3):
                add(f"e{e}.c5.{dh}{dw}", 128, 128)
        add(f"e{e}.fc", 128, 128)
        add(f"e{e}.cls", 128, 10)
    return lay, col


def _blayout():
    lay = {}
    col = 0

    def add(name, rows):
        nonlocal col
        lay[name] = (rows, col)
        col += 1

    add("t.b1", 32)
    add("t.b2", 32)
    add("t.fcb", 64)
    add("t.g1b", 32)
    add("t.g2b", 6)
    for e in range(E):
        for nm, r in [("b1", 32), ("b2", 64), ("b3", 64), ("b4", 128),
                      ("b5", 128), ("fb", 128), ("cb", 10)]:
            add(f"e{e}.{nm}", r)
    return lay, col


WLAY, WCOL = _wlayout()
BLAY, BCOL = _blayout()


# ---------------------------------------------------------------- host prep
def _fold(w, cb, g, bb):
    """Fold conv bias + eval-mode BN into (w', b'). w [co,ci,3,3]."""
    w = np.asarray(w, np.float32)
    g = np.asarray(g, np.float32)
    bb = np.asarray(bb, np.float32)
    s = g / np.sqrt(np.float32(1.0 + BN_EPS))
    wp = w * s[:, None, None, None]
    bp = (np.asarray(cb, np.float32) if cb is not None else 0.0) * s + bb
    return wp.astype(np.float32), np.asarray(bp, np.float32)


def _pack_tap3(wp):
    """[co,ci,3,3] -> lhsT rows (dh*3+dw)*3+ci for the K=27 im2col conv1."""
    return np.ascontiguousarray(wp.transpose(2, 3, 1, 0).reshape(27, wp.shape[0]))


def _pack_96(wp, dw):
    """[co,ci,3,3] -> lhsT [3*ci_n, co] with dh groups ordered DH_ORDER."""
    co, ci = wp.shape[0], wp.shape[1]
    return np.ascontiguousarray(
        wp[:, :, list(DH_ORDER), dw].transpose(2, 1, 0).reshape(3 * ci, co))


def _prep_blobs(inp):
    wb = np.zeros((128, WCOL), np.float32)
    bb = np.zeros((128, BCOL), np.float32)

    def putw(name, arr):
        r, c0, cn = WLAY[name]
        assert arr.shape == (r, cn), (name, arr.shape, (r, cn))
        wb[:r, c0:c0 + cn] = arr

    def putb(name, vec):
        r, c0 = BLAY[name]
        vec = np.asarray(vec, np.float32)
        assert vec.shape == (r,), (name, vec.shape)
        bb[:r, c0] = vec

    # trunk
    w1, b1 = _fold(inp["t_c1w"], None, inp["t_b1g"], inp["t_b1b"])
    putw("t.c1", _pack_tap3(w1)); putb("t.b1", b1)
    w2, b2 = _fold(inp["t_c2w"], None, inp["t_b2g"], inp["t_b2b"])
    for dw in range(3):
        putw(f"t.c2.{dw}", _pack_96(w2, dw))
    putb("t.b2", b2)
    tfcw = np.asarray(inp["t_fcw"], np.float32) / 64.0   # fold avgpool mean
    for k in range(4):
        putw(f"t.fc.{k}", np.ascontiguousarray(tfcw.reshape(64, 32, 4)[:, :, k].T))
    putb("t.fcb", inp["t_fcb"])
    putw("t.g1", np.ascontiguousarray(np.asarray(inp["g1w"], np.float32).T))
    putb("t.g1b", inp["g1b"])
    putw("t.g2", np.ascontiguousarray((np.asarray(inp["g2w"], np.float32) / TEMP).T))
    putb("t.g2b", np.asarray(inp["g2b"], np.float32) / TEMP)

    for e in range(E):
        w1, b1 = _fold(inp["e_c1w"][e], inp["e_c1b"][e], inp["e_b1g"][e], inp["e_b1b"][e])
        putw(f"e{e}.c1", _pack_tap3(w1)); putb(f"e{e}.b1", b1)
        w2, b2 = _fold(inp["e_c2w"][e], inp["e_c2b"][e], inp["e_b2g"][e], inp["e_b2b"][e])
        for dw in range(3):
            putw(f"e{e}.c2.{dw}", _pack_96(w2, dw))
        putb(f"e{e}.b2", b2)
        w3, b3 = _fold(inp["e_c3w"][e], inp["e_c3b"][e], inp["e_b3g"][e], inp["e_b3b"][e])
        for dw in range(3):
            putw(f"e{e}.c3A.{dw}",
                 np.ascontiguousarray(w3[:, :, 0:2, dw].transpose(2, 1, 0).reshape(128, 64)))
            putw(f"e{e}.c3B.{dw}", np.ascontiguousarray(w3[:, :, 2, dw].T))
        putb(f"e{e}.b3", b3)
        w4, b4 = _fold(inp["e_c4w"][e], inp["e_c4b"][e], inp["e_b4g"][e], inp["e_b4b"][e])
        for dw in range(3):
            putw(f"e{e}.c4A.{dw}",
                 np.ascontiguousarray(w4[:, :, 0:2, dw].transpose(2, 1, 0).reshape(128, 128)))
            putw(f"e{e}.c4B.{dw}", np.ascontiguousarray(w4[:, :, 2, dw].T))
        putb(f"e{e}.b4", b4)
        w5, b5 = _fold(inp["e_c5w"][e], inp["e_c5b"][e], inp["e_b5g"][e], inp["e_b5b"][e])
        for dh in range(3):
            for dw in range(3):
                putw(f"e{e}.c5.{dh}{dw}", np.ascontiguousarray(w5[:, :, dh, dw].T))
        putb(f"e{e}.b5", b5)
        putw(f"e{e}.fc",
             np.ascontiguousarray((np.asarray(inp["e_fw"][e], np.float32) / 64.0).T))
        putb(f"e{e}.fb", inp["e_fb"][e])
        putw(f"e{e}.cls", np.ascontiguousarray(np.asarray(inp["e_cw"][e], np.float32).T))
        putb(f"e{e}.cb", inp["e_cb"][e])
    return wb, bb


def _im2col_x(x):
    """x [B,3,32,32] f32 -> [27, B, 1024]: row (dh*3+dw)*3+ci holds
    x[b, ci, h+dh-1, w+dw-1] (zero outside)."""
    Bn = x.shape[0]
    xp = np.zeros((Bn, 3, 34, 34), np.float32)
    xp[:, :, 1:33, 1:33] = x
    out = np.empty((27, Bn, 32, 32), np.float32)
    for dh in range(3):
        for dw in range(3):
            for ci in range(3):
                p = (dh * 3 + dw) * 3 + ci
                out[p] = xp[:, ci, dh:dh + 32, dw:dw + 32]
    return np.ascontiguousarray(out.reshape(27, Bn, 1024))


# ---------------------------------------------------------------- device build
_NC_CACHE = {}


def _build_nc():
    if "nc" in _NC_CACHE:
        return _NC_CACHE["nc"]
    nc = bacc.Bacc("TRN2", target_bir_lowering=False, debug=False)
    x9_d = nc.dram_tensor("x9", [27, BC, 1024], F32R, kind="ExternalInput").ap()
    z_d = nc.dram_tensor("zeros", [128, TB * 32 * 34], F32R, kind="ExternalInput").ap()
    w_d = nc.dram_tensor("wblob", [128, WCOL], F32R, kind="ExternalInput").ap()
    b_d = nc.dram_tensor("bblob", [128, BCOL], F32, kind="ExternalInput").ap()
    gate_d = nc.dram_tensor("gate", [6, BC], F32, kind="ExternalOutput").ap()
    log_d = nc.dram_tensor("logits", [E, 10, BC], F32, kind="ExternalOutput").ap()

    relu = mybir.ActivationFunctionType.Relu
    X = mybir.AxisListType.X
    XY = mybir.AxisListType.XY

    with tile.TileContext(nc) as tc:
        with ExitStack() as ctx:
            wp = ctx.enter_context(tc.tile_pool(name="wpool", bufs=1))
            ap_ = ctx.enter_context(tc.tile_pool(name="acts", bufs=1))
            pp = ctx.enter_context(tc.tile_pool(name="psbig", bufs=6, space="PSUM"))
            ps_ = ctx.enter_context(tc.tile_pool(name="pssmall", bufs=2, space="PSUM"))
            op_ = ctx.enter_context(tc.tile_pool(name="outs", bufs=2))

            wt = wp.tile([128, WCOL], F32R)
            bt = wp.tile([128, BCOL], F32)
            nc.sync.dma_start(wt[:], w_d)
            nc.sync.dma_start(bt[:], b_d)

            def W(name):
                r, c0, cn = WLAY[name]
                return wt[0:r, c0:c0 + cn]

            def BI(name):
                r, c0 = BLAY[name]
                return bt[0:r, c0:c0 + 1]

            xim = ap_.tile([27, TB * 1024], F32R)        # [b,32,32]
            t2in = ap_.tile([96, TB * 32 * 34], F32R)    # [b,32,34] w-padded
            pooled16 = ap_.tile([32, TB * 256], F32)     # [b,16,16]
            avg1 = ap_.tile([32, TB * 32], F32)          # [b,16,2]
            tfeat = ap_.tile([32, 4 * TB], F32R)         # [k(4), b]
            c2a = ap_.tile([96, TB * 288], F32R)         # [b,16,18] ping
            c2b = ap_.tile([96, TB * 288], F32R)         # [b,16,18] pong
            c3in = ap_.tile([128, TB * 324], F32R)       # [b,18,18]
            c4in = ap_.tile([128, TB * 100], F32R)       # [b,10,10]
            c5in = ap_.tile([128, TB * 100], F32R)       # [b,10,10]
            c5out = ap_.tile([128, TB * 64], F32)        # [b,8,8]
            feat = ap_.tile([128, TB], F32R)
            ffeat = ap_.tile([128, TB], F32R)
            rf = ap_.tile([64, TB], F32R)
            gbuf = ap_.tile([32, TB], F32R)

            # zero the padded buffers once (interiors are rewritten every iter)
            for t_ in (xim, t2in, c2a, c2b, c3in, c4in, c5in):
                p_, f_ = t_.shape
                nc.sync.dma_start(t_[:], z_d[0:p_, 0:f_])

            # padded views
            t2v = t2in[:].rearrange("p (b h w) -> p b h w", h=32, w=34)
            p16 = pooled16[:].rearrange("p (b h w) -> p b h w", h=16, w=16)
            a1v = avg1[:].rearrange("p (b h g) -> p b h g", h=16, g=2)
            c2vs = [t[:].rearrange("p (b h w) -> p b h w", h=16, w=18)
                    for t in (c2a, c2b)]
            c3v = c3in[:].rearrange("p (b h w) -> p b h w", h=18, w=18)
            c4v = c4in[:].rearrange("p (b h w) -> p b h w", h=10, w=10)
            c5v = c5in[:].rearrange("p (b h w) -> p b h w", h=10, w=10)
            c5o = c5out[:].rearrange("p (b s) -> p b s", s=64)
            ximv = xim[:].rearrange("p (b s) -> p b s", s=1024)

            def pool2x2(dst, ps, parts):
                """One-op 2x2 maxpool of a [parts, 16x32] psum chunk."""
                v = ps[0:parts, :].rearrange(
                    "p (ho wh wo ww) -> p ho wo wh ww", ho=8, wh=2, ww=2)
                nc.vector.reduce_max(dst, v, axis=XY)

            def conv1_expert(i, e, c2v):
                """Expert conv1 + fused 2x2 maxpool + bias/relu + replicas.

                Finalizes c2v incrementally (per-chunk ACT, per-half replica
                DMAs) so the downstream conv2 never waits on a long chain."""
                for c in range(2 * TB):
                    ic, hf = c // 2, (c % 2) * 16
                    ps = pp.tile([128, 512], F32, tag="big")
                    nc.tensor.matmul(ps[0:32, :], W(f"e{e}.c1"),
                                     xim[:, c * 512:(c + 1) * 512],
                                     start=True, stop=True)
                    reg = c2v[0:32, ic, hf // 2:hf // 2 + 8, 1:17]
                    pool2x2(reg, ps, 32)
                    nc.scalar.activation(reg, reg, relu, bias=BI(f"e{e}.b1"))
                    if c % (TB) == TB - 1:
                        half = c // TB          # images [half*4, half*4+4)
                        b0, b1 = half * (TB // 2), (half + 1) * (TB // 2)
                        nc.sync.dma_start(c2v[32:64, b0:b1, 1:16, :],
                                          c2v[0:32, b0:b1, 0:15, :])
                        nc.sync.dma_start(c2v[64:96, b0:b1, 0:15, :],
                                          c2v[0:32, b0:b1, 1:16, :])

            def trunk_conv1(i):
                for c in range(2 * TB):
                    ic, hf = c // 2, (c % 2) * 16
                    ps = pp.tile([128, 512], F32, tag="big")
                    nc.tensor.matmul(ps[0:32, :], W("t.c1"),
                                     xim[:, c * 512:(c + 1) * 512],
                                     start=True, stop=True)
                    nc.scalar.activation(
                        t2v[0:32, ic, hf:hf + 16, 1:33],
                        ps[0:32, :].rearrange("p (h w) -> p h w", w=32),
                        relu, bias=BI("t.b1"))
                nc.sync.dma_start(t2v[32:64, :, 1:32, :], t2v[0:32, :, 0:31, :])
                nc.sync.dma_start(t2v[64:96, :, 0:31, :], t2v[0:32, :, 1:32, :])

            def trunk_conv2_chunks(i, cs):
                # conv2 (K=96 x 3 dw) + 1-op maxpool -> pooled16 (raw)
                for c in cs:
                    ic, hf = c // 2, (c % 2) * 16
                    ps = pp.tile([128, 512], F32, tag="big")
                    for dw in range(3):
                        nc.tensor.matmul(ps[0:32, :], W(f"t.c2.{dw}"),
                                         t2v[0:96, ic, hf:hf + 16, dw:dw + 32],
                                         start=(dw == 0), stop=(dw == 2))
                    pool2x2(p16[:, ic, hf // 2:hf // 2 + 8, :], ps, 32)

            def trunk_rest(i):
                nc.scalar.activation(pooled16[:], pooled16[:], relu, bias=BI("t.b2"))
                nc.vector.reduce_sum(
                    a1v, p16.rearrange("p b h (g w) -> p b h g w", g=2), axis=X)
                with nc.allow_low_precision(reason="fp32r rounding intended"):
                    nc.vector.reduce_sum(
                        tfeat[:].rearrange("p (i g b) -> p b i g", i=2, g=2),
                        a1v.rearrange("p b (i h) g -> p b i g h", i=2),
                        axis=X)
                psf = ps_.tile([128, TB], F32, tag="small")
                for k in range(4):
                    nc.tensor.matmul(psf[0:64, :], W(f"t.fc.{k}"),
                                     tfeat[:, k * TB:(k + 1) * TB],
                                     start=(k == 0), stop=(k == 3))
                nc.scalar.activation(rf[:], psf[0:64, :], relu, bias=BI("t.fcb"))
                psg = ps_.tile([128, TB], F32, tag="small")
                nc.tensor.matmul(psg[0:32, :], W("t.g1"), rf[:], start=True, stop=True)
                nc.scalar.activation(gbuf[:], psg[0:32, :], relu, bias=BI("t.g1b"))
                psh = ps_.tile([128, TB], F32, tag="small")
                nc.tensor.matmul(psh[0:6, :], W("t.g2"), gbuf[:], start=True, stop=True)
                gs = op_.tile([6, TB], F32, tag="gate")
                nc.vector.tensor_scalar_add(gs[:], psh[0:6, :], BI("t.g2b"))
                nc.sync.dma_start(gate_d[:, bass.ds(i * TB, TB)], gs[:])

            def expert_rest(i, e, c2v):
                """conv2..cls for expert e (c2v already finalized)."""
                # conv2 (K=96 x 3 dw), 2 imgs per chunk -> c3in master interior
                for cc in range(TB // 2):
                    i0 = 2 * cc
                    ps = pp.tile([128, 512], F32, tag="big")
                    for dw in range(3):
                        nc.tensor.matmul(
                            ps[0:64, :], W(f"e{e}.c2.{dw}"),
                            c2v[0:96, i0:i0 + 2, 0:16, dw:dw + 16],
                            start=(dw == 0), stop=(dw == 2))
                    nc.scalar.activation(
                        c3v[0:64, i0:i0 + 2, 1:17, 1:17],
                        ps[0:64, :].rearrange("p (b h w) -> p b h w", b=2, w=16),
                        relu, bias=BI(f"e{e}.b2"))
                # dh=1 replica of pad2: rep rows 0..15 <- master rows 1..16
                nc.sync.dma_start(c3v[64:128, :, 0:16, :], c3v[0:64, :, 1:17, :])

                # conv3 (A: K=128 dh{0,1}; B: K=64 dh=2 via +2-row offset)
                # + 1-op maxpool -> c4in master interior raw
                for cc in range(TB // 2):
                    i0 = 2 * cc
                    ps = pp.tile([128, 512], F32, tag="big")
                    for dw in range(3):
                        nc.tensor.matmul(
                            ps[0:64, :], W(f"e{e}.c3A.{dw}"),
                            c3v[0:128, i0:i0 + 2, 0:16, dw:dw + 16],
                            start=(dw == 0), stop=False)
                    for dw in range(3):
                        nc.tensor.matmul(
                            ps[0:64, :], W(f"e{e}.c3B.{dw}"),
                            c3v[0:64, i0:i0 + 2, 2:18, dw:dw + 16],
                            start=False, stop=(dw == 2))
                    # chunk is [64, 2 imgs x 16 x 16]: pool each img
                    v = ps[0:64, :].rearrange(
                        "p (b ho wh wo ww) -> p b ho wo wh ww",
                        b=2, ho=8, wh=2, ww=2)
                    nc.vector.reduce_max(c4v[0:64, i0:i0 + 2, 1:9, 1:9], v, axis=XY)
                nc.scalar.activation(c4v[0:64, :, 1:9, 1:9],
                                     c4v[0:64, :, 1:9, 1:9],
                                     relu, bias=BI(f"e{e}.b3"))
                nc.sync.dma_start(c4v[64:128, :, 0:8, :], c4v[0:64, :, 1:9, :])

                # conv4 (A: K=128 dh{0,1}; B: K=64 dh=2), one chunk N=512
                ps4 = pp.tile([128, 512], F32, tag="big")
                for dw in range(3):
                    nc.tensor.matmul(ps4[:], W(f"e{e}.c4A.{dw}"),
                                     c4v[0:128, :, 0:8, dw:dw + 8],
                                     start=(dw == 0), stop=False)
                for dw in range(3):
                    nc.tensor.matmul(ps4[:], W(f"e{e}.c4B.{dw}"),
                                     c4v[0:64, :, 2:10, dw:dw + 8],
                                     start=False, stop=(dw == 2))
                nc.scalar.activation(
                    c5v[:, :, 1:9, 1:9],
                    ps4[:].rearrange("p (b h w) -> p b h w", b=TB, w=8),
                    relu, bias=BI(f"e{e}.b4"))

                # conv5 (K=128, 9 taps via offsets), one chunk N=512
                ps5 = pp.tile([128, 512], F32, tag="big")
                for t9 in range(9):
                    dh, dw = t9 // 3, t9 % 3
                    nc.tensor.matmul(ps5[:], W(f"e{e}.c5.{dh}{dw}"),
                                     c5v[:, :, dh:dh + 8, dw:dw + 8],
                                     start=(t9 == 0), stop=(t9 == 8))
                nc.scalar.activation(c5o, ps5[:].rearrange("p (b s) -> p b s", s=64),
                                     relu, bias=BI(f"e{e}.b5"))

                # global average pool (sum; /64 folded into fc) + fc + cls
                with nc.allow_low_precision(reason="fp32r rounding intended"):
                    nc.vector.reduce_sum(feat[:], c5o, axis=X)
                psf2 = ps_.tile([128, TB], F32, tag="small")
                nc.tensor.matmul(psf2[:], W(f"e{e}.fc"), feat[:],
                                 start=True, stop=True)
                nc.scalar.activation(ffeat[:], psf2[:], relu, bias=BI(f"e{e}.fb"))
                psc = ps_.tile([128, TB], F32, tag="small")
                nc.tensor.matmul(psc[0:10, :], W(f"e{e}.cls"), ffeat[:],
                                 start=True, stop=True)
                lg = op_.tile([10, TB], F32, tag="lg")
                nc.vector.tensor_scalar_add(lg[:], psc[0:10, :], BI(f"e{e}.cb"))
                nc.sync.dma_start(log_d[e][:, bass.ds(i * TB, TB)], lg[:])

            def body(i):
                for q in range(4):
                    nc.sync.dma_start(
                        ximv[:, q * 2:q * 2 + 2, :],
                        x9_d[:, bass.ds(i * TB + q * 2, 2), :])
                # software pipeline: conv1(e+1) overlaps expert e's conv2..cls;
                # trunk conv2 chunks sprinkled between experts as PE filler
                conv1_expert(i, 0, c2vs[0])
                trunk_conv1(i)
                tc2 = [list(range(3)), list(range(3, 6)), list(range(6, 9)),
                       list(range(9, 12)), list(range(12, 14)), list(range(14, 16))]
                for e in range(E):
                    if e + 1 < E:
                        conv1_expert(i, e + 1, c2vs[(e + 1) % 2])
                    trunk_conv2_chunks(i, tc2[e])
                    expert_rest(i, e, c2vs[e % 2])
                trunk_rest(i)

            with tc.For_i(0, NIT, 1) as i:
                body(i)

    nc.compile()
    _NC_CACHE["nc"] = nc
    return nc


# ---------------------------------------------------------------- host routing
def _route_and_combine(gate_scores, logits_e, ema):
    """Reproduce the reference's conf/balanced/scan/combine on the host."""
    gate = gate_scores.astype(np.float32)
    lg = logits_e.astype(np.float32)
    # conf = -entropy of per-expert softmax
    m = lg.max(axis=2, keepdims=True)
    z = np.exp(lg - m)
    p = z / z.sum(axis=2, keepdims=True)
    logp = np.log(p + 1e-12)
    conf = (p * logp).sum(axis=2).astype(np.float32)          # [B, E]
    combined = (ALPHA * gate + (1.0 - ALPHA) * conf).astype(np.float32)
    ema = np.asarray(ema, np.float32)
    boost = np.where(ema < MIN_USE, (MIN_USE - ema) * 10.0, 0.0).astype(np.float32)
    balanced = (combined + boost[None, :] - LOAD_PEN * ema[None, :]).astype(np.float32)

    topk = np.argsort(-balanced, axis=1, kind="stable")[:, :KTOP]
    loads = np.zeros(E, np.float32)
    chosen = np.zeros(balanced.shape[0], np.int64)
    for i in range(balanced.shape[0]):
        idxs = topk[i]
        l = loads[idxs]
        fits = l < CAP
        c = idxs[int(np.argmax(fits))] if fits.any() else idxs[int(np.argmin(l))]
        loads[c] += 1.0
        chosen[i] = c
    D = np.zeros((balanced.shape[0], E), bool)
    D[np.arange(balanced.shape[0]), chosen] = True

    masked = np.where(D, balanced, np.float32(NEG)).astype(np.float32)
    mm = masked.max(axis=1, keepdims=True)
    w = np.exp(masked - mm)
    w = w / w.sum(axis=1, keepdims=True)
    w = (w * D).astype(np.float32)
    w = w / (w.sum(axis=1, keepdims=True) + 1e-12)
    logits_final = (w[:, :, None] * lg).sum(axis=1).astype(np.float32)
    return logits_final, balanced, D


def _run_device(inp, trace=False):
    wb, bb = _prep_blobs(inp)
    x9 = _im2col_x(np.asarray(inp["x"], np.float32))
    nc = _build_nc()
    in_maps = []
    for c in range(NCORES):
        in_maps.append({
            "x9": np.ascontiguousarray(x9[:, c * BC:(c + 1) * BC, :]),
            "wblob": wb,
            "bblob": bb,
            "zeros": np.zeros((128, TB * 32 * 34), np.float32),
        })
    res = run_bass_kernel_spmd(nc, in_maps, core_ids=list(range(NCORES)),
                               trace=trace)
    gate_scores = np.empty((B, E), np.float32)
    logits_e = np.empty((B, E, 10), np.float32)
    for c in range(NCORES):
        r = res.results[c]
        gate_scores[c * BC:(c + 1) * BC, :] = r["gate"].T
        logits_e[c * BC:(c + 1) * BC, :, :] = r["logits"].transpose(2, 0, 1)
    return gate_scores, logits_e, res


def kernel(**inputs):
    gate_scores, logits_e, _ = _run_device(inputs, trace=False)
    return _route_and_combine(gate_scores, logits_e, inputs["ema"])


def _install_ntff_hook():
    """Provide antenv.axon_hooks (absent in this image) so trace=True works."""
    import types
    try:
        import antenv.axon_hooks  # noqa: F401
        return
    except ImportError:
        pass
    try:
        import antenv
        sys.path.insert(0, "/root/.axon_site")
        from trn_agent_boot.trn_boot import _ntff_profile_via_ctypes
        mod = types.ModuleType("antenv.axon_hooks")
        _h = [None]
        mod.set_axon_ntff_profile_hook = lambda h: _h.__setitem__(0, h)
        mod.get_axon_ntff_profile_hook = lambda: _h[0]
        sys.modules["antenv.axon_hooks"] = mod
        antenv.axon_hooks = mod
        mod.set_axon_ntff_profile_hook(
            _ntff_profile_via_ctypes("/opt/axon/libaxon_pjrt.so"))
    except Exception as ex:  # profiling is best-effort
        print("ntff hook install failed:", ex)


def kernel_with_stats(**inputs):
    """Like kernel() but traces the run and returns (outputs, exec_time_ns)."""
    _install_ntff_hook()
    gate_scores, logits_e, res = _run_device(inputs, trace=True)
    out = _route_and_combine(gate_scores, logits_e, inputs["ema"])
    return out, res.exec_time_ns


# revision 14
# speedup vs baseline: 1.7810x; 1.3585x over previous
"""Trainium2 Bass kernel for nn_DistributedMoE (moe_routing).

Strategy: pure data-parallel over batch across 8 NeuronCores (128 images each).
Each core runs the routing trunk + gate + all 6 expert CNNs on its slice.
All convs run as shifted-window matmuls on the PE in float32r
(~1.6e-4 rel err, 4x the throughput of plain fp32):
  - conv1 (Cin=3): host-side im2col to K=27 (taps x channels on partitions).
  - conv2 (Cin=32): 3 dh-tap groups packed into K=96. Partition-group order is
    [dh=1, dh=0, dh=2] so the full-data group sits at partition 0 (engines
    cannot shift partitions; the dh=0/dh=2 row-shifted replicas are made by
    SBUF->SBUF DMA, which can).
  - conv3/conv4 (Cin=64): dh {0,1} packed into K=128 (master = fully padded
    buffer at partitions 0:64, dh=1 replica at 64:128) + dh=2 via a +2-row
    free-offset read of the master (K=64).
  - conv5 (Cin=128): 9 taps as free-dim offsets into a padded buffer.
BatchNorm is folded into conv weights/biases on the host; bias+ReLU fuse into
the ScalarE activation pass that drains PSUM; 2x2 maxpools run as
reduce_max (PSUM, w-pairs) + tensor_max (SBUF, h-pairs) on VectorE.
The device returns gate scores [6,B] and per-expert logits [6,10,B]; the tiny
sequential capacity-constrained routing scan + combine run on the host.
"""

import sys

import numpy as np

sys.path.insert(0, "/opt/trn_rl_repo")

from contextlib import ExitStack

import concourse.bacc as bacc
import concourse.bass as bass
import concourse.mybir as mybir
import concourse.tile as tile
from concourse.bass_utils import run_bass_kernel_spmd

# problem constants (from the reference)
E, KTOP, CAP = 6, 2, 192
ALPHA, LOAD_PEN, MIN_USE, TEMP = 0.7, 2.0, 0.05, 1.0
BN_EPS = 1e-5
NEG = -1e30

B = 1024
NCORES = 8
BC = B // NCORES          # images per core
TB = 8                    # images per tile
NIT = BC // TB            # loop iterations per core

F32 = mybir.dt.float32
F32R = mybir.dt.float32r

# dh order for K=96 packing: full-data group (dh=1) first (partition base 0)
DH_ORDER = (1, 0, 2)


# ---------------------------------------------------------------- layouts
def _wlayout():
    """Weight-blob column layout: name -> (rows, col_start, cols)."""
    lay = {}
    col = 0

    def add(name, rows, cols):
        nonlocal col
        lay[name] = (rows, col, cols)
        col += cols

    add("t.c1", 27, 32)
    for dw in range(3):
        add(f"t.c2.{dw}", 96, 32)
    for k in range(4):
        add(f"t.fc.{k}", 32, 64)
    add("t.g1", 64, 32)
    add("t.g2", 32, 6)
    for e in range(E):
        add(f"e{e}.c1", 27, 32)
        for dw in range(3):
            add(f"e{e}.c2.{dw}", 96, 64)
        for dw in range(3):
            add(f"e{e}.c3A.{dw}", 128, 64)
        for dw in range(3):
            add(f"e{e}.c3B.{dw}", 64, 64)
        for dw in range(3):
            add(f"e{e}.c4A.{dw}", 128, 128)
        for dw in range(3):
            add(f"e{e}.c4B.{dw}", 64, 128)
        for dh in range(3):
            for dw in range(3):
                add(f"e{e}.c5.{dh}{dw}", 128, 128)
        add(f"e{e}.fc", 128, 128)
        add(f"e{e}.cls", 128, 10)
    return lay, col


def _blayout():
    lay = {}
    col = 0

    def add(name, rows):
        nonlocal col
        lay[name] = (rows, col)
        col += 1

    add("t.b1", 32)
    add("t.b2", 32)
    add("t.fcb", 64)
    add("t.g1b", 32)
    add("t.g2b", 6)
    for e in range(E):
        for nm, r in [("b1", 32), ("b2", 64), ("b3", 64), ("b4", 128),
                      ("b5", 128), ("fb", 128), ("cb", 10)]:
            add(f"e{e}.{nm}", r)
    return lay, col


WLAY, WCOL = _wlayout()
BLAY, BCOL = _blayout()


# ---------------------------------------------------------------- host prep
def _fold(w, cb, g, bb):
    """Fold conv bias + eval-mode BN into (w', b'). w [co,ci,3,3]."""
    w = np.asarray(w, np.float32)
    g = np.asarray(g, np.float32)
    bb = np.asarray(bb, np.float32)
    s = g / np.sqrt(np.float32(1.0 + BN_EPS))
    wp = w * s[:, None, None, None]
    bp = (np.asarray(cb, np.float32) if cb is not None else 0.0) * s + bb
    return wp.astype(np.float32), np.asarray(bp, np.float32)


def _pack_tap3(wp):
    """[co,ci,3,3] -> lhsT rows (dh*3+dw)*3+ci for the K=27 im2col conv1."""
    return np.ascontiguousarray(wp.transpose(2, 3, 1, 0).reshape(27, wp.shape[0]))


def _pack_96(wp, dw):
    """[co,ci,3,3] -> lhsT [3*ci_n, co] with dh groups ordered DH_ORDER."""
    co, ci = wp.shape[0], wp.shape[1]
    return np.ascontiguousarray(
        wp[:, :, list(DH_ORDER), dw].transpose(2, 1, 0).reshape(3 * ci, co))


def _prep_blobs(inp):
    wb = np.zeros((128, WCOL), np.float32)
    bb = np.zeros((128, BCOL), np.float32)

    def putw(name, arr):
        r, c0, cn = WLAY[name]
        assert arr.shape == (r, cn), (name, arr.shape, (r, cn))
        wb[:r, c0:c0 + cn] = arr

    def putb(name, vec):
        r, c0 = BLAY[name]
        vec = np.asarray(vec, np.float32)
        assert vec.shape == (r,), (name, vec.shape)
        bb[:r, c0] = vec

    # trunk
    w1, b1 = _fold(inp["t_c1w"], None, inp["t_b1g"], inp["t_b1b"])
    putw("t.c1", _pack_tap3(w1)); putb("t.b1", b1)
    w2, b2 = _fold(inp["t_c2w"], None, inp["t_b2g"], inp["t_b2b"])
    for dw in range(3):
        putw(f"t.c2.{dw}", _pack_96(w2, dw))
    putb("t.b2", b2)
    tfcw = np.asarray(inp["t_fcw"], np.float32) / 64.0   # fold avgpool mean
    for k in range(4):
        putw(f"t.fc.{k}", np.ascontiguousarray(tfcw.reshape(64, 32, 4)[:, :, k].T))
    putb("t.fcb", inp["t_fcb"])
    putw("t.g1", np.ascontiguousarray(np.asarray(inp["g1w"], np.float32).T))
    putb("t.g1b", inp["g1b"])
    putw("t.g2", np.ascontiguousarray((np.asarray(inp["g2w"], np.float32) / TEMP).T))
    putb("t.g2b", np.asarray(inp["g2b"], np.float32) / TEMP)

    for e in range(E):
        w1, b1 = _fold(inp["e_c1w"][e], inp["e_c1b"][e], inp["e_b1g"][e], inp["e_b1b"][e])
        putw(f"e{e}.c1", _pack_tap3(w1)); putb(f"e{e}.b1", b1)
        w2, b2 = _fold(inp["e_c2w"][e], inp["e_c2b"][e], inp["e_b2g"][e], inp["e_b2b"][e])
        for dw in range(3):
            putw(f"e{e}.c2.{dw}", _pack_96(w2, dw))
        putb(f"e{e}.b2", b2)
        w3, b3 = _fold(inp["e_c3w"][e], inp["e_c3b"][e], inp["e_b3g"][e], inp["e_b3b"][e])
        for dw in range(3):
            putw(f"e{e}.c3A.{dw}",
                 np.ascontiguousarray(w3[:, :, 0:2, dw].transpose(2, 1, 0).reshape(128, 64)))
            putw(f"e{e}.c3B.{dw}", np.ascontiguousarray(w3[:, :, 2, dw].T))
        putb(f"e{e}.b3", b3)
        w4, b4 = _fold(inp["e_c4w"][e], inp["e_c4b"][e], inp["e_b4g"][e], inp["e_b4b"][e])
        for dw in range(3):
            putw(f"e{e}.c4A.{dw}",
                 np.ascontiguousarray(w4[:, :, 0:2, dw].transpose(2, 1, 0).reshape(128, 128)))
            putw(f"e{e}.c4B.{dw}", np.ascontiguousarray(w4[:, :, 2, dw].T))
        putb(f"e{e}.b4", b4)
        w5, b5 = _fold(inp["e_c5w"][e], inp["e_c5b"][e], inp["e_b5g"][e], inp["e_b5b"][e])
        for dh in range(3):
            for dw in range(3):
                putw(f"e{e}.c5.{dh}{dw}", np.ascontiguousarray(w5[:, :, dh, dw].T))
        putb(f"e{e}.b5", b5)
        putw(f"e{e}.fc",
             np.ascontiguousarray((np.asarray(inp["e_fw"][e], np.float32) / 64.0).T))
        putb(f"e{e}.fb", inp["e_fb"][e])
        putw(f"e{e}.cls", np.ascontiguousarray(np.asarray(inp["e_cw"][e], np.float32).T))
        putb(f"e{e}.cb", inp["e_cb"][e])
    return wb, bb


def _im2col_x(x):
    """x [B,3,32,32] f32 -> [27, B, 1024]: row (dh*3+dw)*3+ci holds
    x[b, ci, h+dh-1, w+dw-1] (zero outside)."""
    Bn = x.shape[0]
    xp = np.zeros((Bn, 3, 34, 34), np.float32)
    xp[:, :, 1:33, 1:33] = x
    out = np.empty((27, Bn, 32, 32), np.float32)
    for dh in range(3):
        for dw in range(3):
            for ci in range(3):
                p = (dh * 3 + dw) * 3 + ci
                out[p] = xp[:, ci, dh:dh + 32, dw:dw + 32]
    return np.ascontiguousarray(out.reshape(27, Bn, 1024))


# ---------------------------------------------------------------- device build
_NC_CACHE = {}


def _build_nc():
    if "nc" in _NC_CACHE:
        return _NC_CACHE["nc"]
    nc = bacc.Bacc("TRN2", target_bir_lowering=False, debug=False)
    x9_d = nc.dram_tensor("x9", [27, BC, 1024], F32R, kind="ExternalInput").ap()
    z_d = nc.dram_tensor("zeros", [128, TB * 32 * 34], F32R, kind="ExternalInput").ap()
    w_d = nc.dram_tensor("wblob", [128, WCOL], F32R, kind="ExternalInput").ap()
    b_d = nc.dram_tensor("bblob", [128, BCOL], F32, kind="ExternalInput").ap()
    gate_d = nc.dram_tensor("gate", [6, BC], F32, kind="ExternalOutput").ap()
    log_d = nc.dram_tensor("logits", [E, 10, BC], F32, kind="ExternalOutput").ap()

    relu = mybir.ActivationFunctionType.Relu
    X = mybir.AxisListType.X
    XY = mybir.AxisListType.XY

    with tile.TileContext(nc) as tc:
        with ExitStack() as ctx:
            wp = ctx.enter_context(tc.tile_pool(name="wpool", bufs=1))
            ap_ = ctx.enter_context(tc.tile_pool(name="acts", bufs=1))
            pp = ctx.enter_context(tc.tile_pool(name="psbig", bufs=6, space="PSUM"))
            ps_ = ctx.enter_context(tc.tile_pool(name="pssmall", bufs=2, space="PSUM"))
            op_ = ctx.enter_context(tc.tile_pool(name="outs", bufs=2))

            wt = wp.tile([128, WCOL], F32R)
            bt = wp.tile([128, BCOL], F32)
            nc.sync.dma_start(wt[:], w_d)
            nc.sync.dma_start(bt[:], b_d)

            def W(name):
                r, c0, cn = WLAY[name]
                return wt[0:r, c0:c0 + cn]

            def BI(name):
                r, c0 = BLAY[name]
                return bt[0:r, c0:c0 + 1]

            xim = ap_.tile([27, TB * 1024], F32R)        # [b,32,32]
            t2in = ap_.tile([96, TB * 32 * 34], F32R)    # [b,32,34] w-padded
            pooled16 = ap_.tile([32, TB * 256], F32)     # [b,16,16]
            avg1 = ap_.tile([32, TB * 32], F32)          # [b,16,2]
            tfeat = ap_.tile([32, 4 * TB], F32R)         # [k(4), b]
            c2a = ap_.tile([96, TB * 288], F32R)         # [b,16,18] ping
            c2b = ap_.tile([96, TB * 288], F32R)         # [b,16,18] pong
            c3in = ap_.tile([128, TB * 324], F32R)       # [b,18,18]
            c4in = ap_.tile([128, TB * 100], F32R)       # [b,10,10]
            c5in = ap_.tile([128, TB * 100], F32R)       # [b,10,10]
            c5out = ap_.tile([128, TB * 64], F32)        # [b,8,8]
            feat = ap_.tile([128, TB], F32R)
            ffeat = ap_.tile([128, TB], F32R)
            rf = ap_.tile([64, TB], F32R)
            gbuf = ap_.tile([32, TB], F32R)

            # zero the padded buffers once (interiors are rewritten every iter)
            for t_ in (xim, t2in, c2a, c2b, c3in, c4in, c5in):
                p_, f_ = t_.shape
                nc.sync.dma_start(t_[:], z_d[0:p_, 0:f_])

            # padded views
            t2v = t2in[:].rearrange("p (b h w) -> p b h w", h=32, w=34)
            p16 = pooled16[:].rearrange("p (b h w) -> p b h w", h=16, w=16)
            a1v = avg1[:].rearrange("p (b h g) -> p b h g", h=16, g=2)
            c2vs = [t[:].rearrange("p (b h w) -> p b h w", h=16, w=18)
                    for t in (c2a, c2b)]
            c3v = c3in[:].rearrange("p (b h w) -> p b h w", h=18, w=18)
            c4v = c4in[:].rearrange("p (b h w) -> p b h w", h=10, w=10)
            c5v = c5in[:].rearrange("p (b h w) -> p b h w", h=10, w=10)
            c5o = c5out[:].rearrange("p (b s) -> p b s", s=64)
            ximv = xim[:].rearrange("p (b s) -> p b s", s=1024)

            def pool2x2(dst, ps, parts):
                """One-op 2x2 maxpool of a [parts, 16x32] psum chunk."""
                v = ps[0:parts, :].rearrange(
                    "p (ho wh wo ww) -> p ho wo wh ww", ho=8, wh=2, ww=2)
                nc.vector.reduce_max(dst, v, axis=XY)

            def conv1_expert(i, e, c2v, cs=None):
                """Expert conv1 + fused 2x2 maxpool + bias/relu + replicas.

                Finalizes c2v incrementally (per-chunk ACT, per-half replica
                DMAs) so the downstream conv2 never waits on a long chain."""
                for c in (cs if cs is not None else range(2 * TB)):
                    ic, hf = c // 2, (c % 2) * 16
                    ps = pp.tile([128, 512], F32, tag="big")
                    nc.tensor.matmul(ps[0:32, :], W(f"e{e}.c1"),
                                     xim[:, c * 512:(c + 1) * 512],
                                     start=True, stop=True)
                    reg = c2v[0:32, ic, hf // 2:hf // 2 + 8, 1:17]
                    pool2x2(reg, ps, 32)
                    nc.scalar.activation(reg, reg, relu, bias=BI(f"e{e}.b1"))
                    if c % (TB) == TB - 1:
                        half = c // TB          # images [half*4, half*4+4)
                        b0, b1 = half * (TB // 2), (half + 1) * (TB // 2)
                        nc.sync.dma_start(c2v[32:64, b0:b1, 1:16, :],
                                          c2v[0:32, b0:b1, 0:15, :])
                        nc.sync.dma_start(c2v[64:96, b0:b1, 0:15, :],
                                          c2v[0:32, b0:b1, 1:16, :])

            def trunk_conv1(i):
                for c in range(2 * TB):
                    ic, hf = c // 2, (c % 2) * 16
                    ps = pp.tile([128, 512], F32, tag="big")
                    nc.tensor.matmul(ps[0:32, :], W("t.c1"),
                                     xim[:, c * 512:(c + 1) * 512],
                                     start=True, stop=True)
                    nc.scalar.activation(
                        t2v[0:32, ic, hf:hf + 16, 1:33],
                        ps[0:32, :].rearrange("p (h w) -> p h w", w=32),
                        relu, bias=BI("t.b1"))
                nc.sync.dma_start(t2v[32:64, :, 1:32, :], t2v[0:32, :, 0:31, :])
                nc.sync.dma_start(t2v[64:96, :, 0:31, :], t2v[0:32, :, 1:32, :])

            def trunk_conv2_chunks(i, cs):
                # conv2 (K=96 x 3 dw) + 1-op maxpool -> pooled16 (raw)
                for c in cs:
                    ic, hf = c // 2, (c % 2) * 16
                    ps = pp.tile([128, 512], F32, tag="big")
                    for dw in range(3):
                        nc.tensor.matmul(ps[0:32, :], W(f"t.c2.{dw}"),
                                         t2v[0:96, ic, hf:hf + 16, dw:dw + 32],
                                         start=(dw == 0), stop=(dw == 2))
                    pool2x2(p16[:, ic, hf // 2:hf // 2 + 8, :], ps, 32)

            def trunk_rest(i):
                nc.scalar.activation(pooled16[:], pooled16[:], relu, bias=BI("t.b2"))
                nc.vector.reduce_sum(
                    a1v, p16.rearrange("p b h (g w) -> p b h g w", g=2), axis=X)
                with nc.allow_low_precision(reason="fp32r rounding intended"):
                    nc.vector.reduce_sum(
                        tfeat[:].rearrange("p (i g b) -> p b i g", i=2, g=2),
                        a1v.rearrange("p b (i h) g -> p b i g h", i=2),
                        axis=X)
                psf = ps_.tile([128, TB], F32, tag="small")
                for k in range(4):
                    nc.tensor.matmul(psf[0:64, :], W(f"t.fc.{k}"),
                                     tfeat[:, k * TB:(k + 1) * TB],
                                     start=(k == 0), stop=(k == 3))
                nc.scalar.activation(rf[:], psf[0:64, :], relu, bias=BI("t.fcb"))
                psg = ps_.tile([128, TB], F32, tag="small")
                nc.tensor.matmul(psg[0:32, :], W("t.g1"), rf[:], start=True, stop=True)
                nc.scalar.activation(gbuf[:], psg[0:32, :], relu, bias=BI("t.g1b"))
                psh = ps_.tile([128, TB], F32, tag="small")
                nc.tensor.matmul(psh[0:6, :], W("t.g2"), gbuf[:], start=True, stop=True)
                gs = op_.tile([6, TB], F32, tag="gate")
                nc.vector.tensor_scalar_add(gs[:], psh[0:6, :], BI("t.g2b"))
                nc.sync.dma_start(gate_d[:, bass.ds(i * TB, TB)], gs[:])

            def conv2_expert(i, e, c2v):
                # conv2 (K=96 x 3 dw), 2 imgs per chunk -> c3in master interior
                # with per-chunk dh=1 replica (rep rows 0..15 <- master 1..16)
                for cc in range(TB // 2):
                    i0 = 2 * cc
                    ps = pp.tile([128, 512], F32, tag="big")
                    for dw in range(3):
                        nc.tensor.matmul(
                            ps[0:64, :], W(f"e{e}.c2.{dw}"),
                            c2v[0:96, i0:i0 + 2, 0:16, dw:dw + 16],
                            start=(dw == 0), stop=(dw == 2))
                    nc.scalar.activation(
                        c3v[0:64, i0:i0 + 2, 1:17, 1:17],
                        ps[0:64, :].rearrange("p (b h w) -> p b h w", b=2, w=16),
                        relu, bias=BI(f"e{e}.b2"))
                    nc.sync.dma_start(c3v[64:128, i0:i0 + 2, 0:16, :],
                                      c3v[0:64, i0:i0 + 2, 1:17, :])

            def conv3_expert(i, e):
                # conv3 (A: K=128 dh{0,1}; B: K=64 dh=2 via +2-row offset)
                # + 1-op maxpool + per-chunk bias/relu + per-chunk replica
                for cc in range(TB // 2):
                    i0 = 2 * cc
                    ps = pp.tile([128, 512], F32, tag="big")
                    for dw in range(3):
                        nc.tensor.matmul(
                            ps[0:64, :], W(f"e{e}.c3A.{dw}"),
                            c3v[0:128, i0:i0 + 2, 0:16, dw:dw + 16],
                            start=(dw == 0), stop=False)
                    for dw in range(3):
                        nc.tensor.matmul(
                            ps[0:64, :], W(f"e{e}.c3B.{dw}"),
                            c3v[0:64, i0:i0 + 2, 2:18, dw:dw + 16],
                            start=False, stop=(dw == 2))
                    v = ps[0:64, :].rearrange(
                        "p (b ho wh wo ww) -> p b ho wo wh ww",
                        b=2, ho=8, wh=2, ww=2)
                    reg = c4v[0:64, i0:i0 + 2, 1:9, 1:9]
                    nc.vector.reduce_max(reg, v, axis=XY)
                    nc.scalar.activation(reg, reg, relu, bias=BI(f"e{e}.b3"))
                    nc.sync.dma_start(c4v[64:128, i0:i0 + 2, 0:8, :],
                                      c4v[0:64, i0:i0 + 2, 1:9, :])

            def conv45_expert(i, e):
                # conv4 (A: K=128 dh{0,1}; B: K=64 dh=2), one chunk N=512
                ps4 = pp.tile([128, 512], F32, tag="big")
                for dw in range(3):
                    nc.tensor.matmul(ps4[:], W(f"e{e}.c4A.{dw}"),
                                     c4v[0:128, :, 0:8, dw:dw + 8],
                                     start=(dw == 0), stop=False)
                for dw in range(3):
                    nc.tensor.matmul(ps4[:], W(f"e{e}.c4B.{dw}"),
                                     c4v[0:64, :, 2:10, dw:dw + 8],
                                     start=False, stop=(dw == 2))
                nc.scalar.activation(
                    c5v[:, :, 1:9, 1:9],
                    ps4[:].rearrange("p (b h w) -> p b h w", b=TB, w=8),
                    relu, bias=BI(f"e{e}.b4"))

                # conv5 (K=128, 9 taps via offsets), one chunk N=512
                ps5 = pp.tile([128, 512], F32, tag="big")
                for t9 in range(9):
                    dh, dw = t9 // 3, t9 % 3
                    nc.tensor.matmul(ps5[:], W(f"e{e}.c5.{dh}{dw}"),
                                     c5v[:, :, dh:dh + 8, dw:dw + 8],
                                     start=(t9 == 0), stop=(t9 == 8))
                nc.scalar.activation(c5o, ps5[:].rearrange("p (b s) -> p b s", s=64),
                                     relu, bias=BI(f"e{e}.b5"))

                # global average pool (sum; /64 folded into fc) + fc + cls
                with nc.allow_low_precision(reason="fp32r rounding intended"):
                    nc.vector.reduce_sum(feat[:], c5o, axis=X)
                psf2 = ps_.tile([128, TB], F32, tag="small")
                nc.tensor.matmul(psf2[:], W(f"e{e}.fc"), feat[:],
                                 start=True, stop=True)
                nc.scalar.activation(ffeat[:], psf2[:], relu, bias=BI(f"e{e}.fb"))
                psc = ps_.tile([128, TB], F32, tag="small")
                nc.tensor.matmul(psc[0:10, :], W(f"e{e}.cls"), ffeat[:],
                                 start=True, stop=True)
                lg = op_.tile([10, TB], F32, tag="lg")
                nc.vector.tensor_scalar_add(lg[:], psc[0:10, :], BI(f"e{e}.cb"))
                nc.sync.dma_start(log_d[e][:, bass.ds(i * TB, TB)], lg[:])

            def body(i):
                for q in range(4):
                    nc.sync.dma_start(
                        ximv[:, q * 2:q * 2 + 2, :],
                        x9_d[:, bass.ds(i * TB + q * 2, 2), :])
                # software pipeline: conv1(e+1) and trunk conv2 emitted as
                # static PE filler INSIDE expert e's layer-transition gaps
                conv1_expert(i, 0, c2vs[0])
                trunk_conv1(i)
                tc2 = [list(range(3)), list(range(3, 6)), list(range(6, 9)),
                       list(range(9, 12)), list(range(12, 14)), list(range(14, 16))]
                for e in range(E):
                    nxt = e + 1 if e + 1 < E else None
                    conv2_expert(i, e, c2vs[e % 2])
                    if nxt is not None:
                        conv1_expert(i, nxt, c2vs[nxt % 2], range(0, TB))
                    conv3_expert(i, e)
                    if nxt is not None:
                        conv1_expert(i, nxt, c2vs[nxt % 2], range(TB, 2 * TB))
                    trunk_conv2_chunks(i, tc2[e])
                    conv45_expert(i, e)
                trunk_rest(i)

            with tc.For_i(0, NIT, 1) as i:
                body(i)

    nc.compile()
    _NC_CACHE["nc"] = nc
    return nc


# ---------------------------------------------------------------- host routing
def _route_and_combine(gate_scores, logits_e, ema):
    """Reproduce the reference's conf/balanced/scan/combine on the host."""
    gate = gate_scores.astype(np.float32)
    lg = logits_e.astype(np.float32)
    # conf = -entropy of per-expert softmax
    m = lg.max(axis=2, keepdims=True)
    z = np.exp(lg - m)
    p = z / z.sum(axis=2, keepdims=True)
    logp = np.log(p + 1e-12)
    conf = (p * logp).sum(axis=2).astype(np.float32)          # [B, E]
    combined = (ALPHA * gate + (1.0 - ALPHA) * conf).astype(np.float32)
    ema = np.asarray(ema, np.float32)
    boost = np.where(ema < MIN_USE, (MIN_USE - ema) * 10.0, 0.0).astype(np.float32)
    balanced = (combined + boost[None, :] - LOAD_PEN * ema[None, :]).astype(np.float32)

    topk = np.argsort(-balanced, axis=1, kind="stable")[:, :KTOP]
    loads = np.zeros(E, np.float32)
    chosen = np.zeros(balanced.shape[0], np.int64)
    for i in range(balanced.shape[0]):
        idxs = topk[i]
        l = loads[idxs]
        fits = l < CAP
        c = idxs[int(np.argmax(fits))] if fits.any() else idxs[int(np.argmin(l))]
        loads[c] += 1.0
        chosen[i] = c
    D = np.zeros((balanced.shape[0], E), bool)
    D[np.arange(balanced.shape[0]), chosen] = True

    masked = np.where(D, balanced, np.float32(NEG)).astype(np.float32)
    mm = masked.max(axis=1, keepdims=True)
    w = np.exp(masked - mm)
    w = w / w.sum(axis=1, keepdims=True)
    w = (w * D).astype(np.float32)
    w = w / (w.sum(axis=1, keepdims=True) + 1e-12)
    logits_final = (w[:, :, None] * lg).sum(axis=1).astype(np.float32)
    return logits_final, balanced, D


def _run_device(inp, trace=False):
    wb, bb = _prep_blobs(inp)
    x9 = _im2col_x(np.asarray(inp["x"], np.float32))
    nc = _build_nc()
    in_maps = []
    for c in range(NCORES):
        in_maps.append({
            "x9": np.ascontiguousarray(x9[:, c * BC:(c + 1) * BC, :]),
            "wblob": wb,
            "bblob": bb,
            "zeros": np.zeros((128, TB * 32 * 34), np.float32),
        })
    res = run_bass_kernel_spmd(nc, in_maps, core_ids=list(range(NCORES)),
                               trace=trace)
    gate_scores = np.empty((B, E), np.float32)
    logits_e = np.empty((B, E, 10), np.float32)
    for c in range(NCORES):
        r = res.results[c]
        gate_scores[c * BC:(c + 1) * BC, :] = r["gate"].T
        logits_e[c * BC:(c + 1) * BC, :, :] = r["logits"].transpose(2, 0, 1)
    return gate_scores, logits_e, res


def kernel(**inputs):
    gate_scores, logits_e, _ = _run_device(inputs, trace=False)
    return _route_and_combine(gate_scores, logits_e, inputs["ema"])


def _install_ntff_hook():
    """Provide antenv.axon_hooks (absent in this image) so trace=True works."""
    import types
    try:
        import antenv.axon_hooks  # noqa: F401
        return
    except ImportError:
        pass
    try:
        import antenv
        sys.path.insert(0, "/root/.axon_site")
        from trn_agent_boot.trn_boot import _ntff_profile_via_ctypes
        mod = types.ModuleType("antenv.axon_hooks")
        _h = [None]
        mod.set_axon_ntff_profile_hook = lambda h: _h.__setitem__(0, h)
        mod.get_axon_ntff_profile_hook = lambda: _h[0]
        sys.modules["antenv.axon_hooks"] = mod
        antenv.axon_hooks = mod
        mod.set_axon_ntff_profile_hook(
            _ntff_profile_via_ctypes("/opt/axon/libaxon_pjrt.so"))
    except Exception as ex:  # profiling is best-effort
        print("ntff hook install failed:", ex)


def kernel_with_stats(**inputs):
    """Like kernel() but traces the run and returns (outputs, exec_time_ns)."""
    _install_ntff_hook()
    gate_scores, logits_e, res = _run_device(inputs, trace=True)
    out = _route_and_combine(gate_scores, logits_e, inputs["ema"])
    return out, res.exec_time_ns
